# revision 1
# baseline (speedup 1.0000x reference)
"""Multi-head attention V2 kernel for Trainium2 (8 NeuronCores).

Problem shapes (hardcoded): x [4, 2048, 512] f32, Wq [512, 4096], Wv unused,
Wp [4096, 512], bp [512].  Reference math (note: V uses the Q projection):
    q = v = (x @ Wq) -> [B, H, N, D] with H=8, head dim = D = 512
    S = q @ x^T / sqrt(D);  P = softmax(S, -1);  out = (P @ v) @ Wp + bp

Sharding: core = (batch b, head-group hg) with 2 groups of 4 heads.
Each core gets x[b]^T and the Wq columns / Wp rows of its 4 heads, computes
its partial output [N, D]; host sums the two head-group partials per batch
and adds the bias.

Per-core kernel.  The scores matmul S^T = x q^T runs in fp8e4 DoubleRow
mode (2 fp8 MACs / PE cell / cycle, K=256 per instruction): both operands
are quantized to TRN fp8_e4m3 (xT8 host-converted from fp32; qT8 converted
on-device from the fp16 qT produced by DMA xbar transposes).  Measured
end-to-end numerics (fp64 reference, these exact inputs): rel err 1.86e-2
vs the 2e-2 gate.  Everything else stays fp16 (fp32 PSUM accumulation).

The emission is software-pipelined so the PE never waits on ScalarE's exp
(688ns per [128,512] tile vs 430ns for a DoubleRow score pair): each
phase interleaves chunk c's score pairs (and stage-B tiles at head
boundaries) with chunk c-1's AV matmuls, 1 pair : 4 AV MMs per slot, so
exp always trails into slack.  Denominator matmuls (ones^T expS, packed
4-way into PE column groups) lag their exps by >=2 slots inside the same
phase; the K=4 sum+broadcast matmul, reciprocal, and the outT normalize
all complete during the next phase before their consumers.

Phase sequence: B(0); S(0,0..3); B(1)+AV(0,3); S(1,0)+<none>; ... with
AV(h,c) consumed by the phase following S(h,c).  Head 3 additionally
carries the final projection y = sum_h outT_h^T Wp_h (region r_c after
AV(3,c)'s normalize), with the tail regions in two flush phases.
"""

import sys

sys.path.insert(0, "/opt/trn_rl_repo")

import numpy as np
import ml_dtypes

B, N, D, H = 4, 2048, 512, 8
NCORES = 8
HG = 2            # head groups (cores per batch)
HPG = H // HG     # heads per core
JW = HPG * D      # per-core Wq column count / Wp row count (2048)
KT = D // 128     # k-tiles over feature dim (4)
NT = N // 128     # partition tiles over tokens (16)
NCHUNK = 4        # n split into 4 chunks of 512
CW = N // NCHUNK  # chunk width (512)
INV_SQRT_D = 1.0 / float(np.sqrt(D))

_state = {}


def _build():
    import concourse.bass as bass
    import concourse.mybir as mybir
    import concourse.tile as tile
    from concourse import bacc

    f32 = mybir.dt.float32
    bf16 = mybir.dt.float16
    f8 = mybir.dt.float8e4
    DR = mybir.MatmulPerfMode.DoubleRow

    nc = bacc.Bacc("TRN2", target_bir_lowering=False)

    xT_d = nc.dram_tensor("xt", [D, N], bf16, kind="ExternalInput")
    xT8_d = nc.dram_tensor("xt8", [D, N], f8, kind="ExternalInput")
    wq_d = nc.dram_tensor("wq", [D, JW], bf16, kind="ExternalInput")
    wp_d = nc.dram_tensor("wp", [JW, D], bf16, kind="ExternalInput")
    y_d = nc.dram_tensor("y", [N, D], f32, kind="ExternalOutput")

    with tile.TileContext(nc) as tc:
        with (
            tc.tile_pool(name="const", bufs=1) as cpool,
            tc.tile_pool(name="qts", bufs=4) as qts_pool,
            tc.tile_pool(name="qt8", bufs=1) as qt8_pool,
            tc.tile_pool(name="qn", bufs=2) as qn_pool,
            tc.tile_pool(name="exps", bufs=2) as exps_pool,
            tc.tile_pool(name="outt", bufs=1) as outt_pool,
            tc.tile_pool(name="rcp", bufs=2) as rcp_pool,
            tc.tile_pool(name="ysb", bufs=2) as y_pool,
            tc.tile_pool(name="ps_stage", bufs=2, space="PSUM") as ps_stage,
            tc.tile_pool(name="ps_scores", bufs=3, space="PSUM") as ps_scores,
            tc.tile_pool(name="ps_av", bufs=2, space="PSUM") as ps_av,
            tc.tile_pool(name="ps_den", bufs=1, space="PSUM") as ps_den,
        ):
            # ---- resident inputs ----
            xT = cpool.tile([128, KT, N], bf16, name="xT")
            xT8 = cpool.tile([128, KT, N], f8, name="xT8")
            wq = cpool.tile([128, KT, JW], bf16, name="wq")
            wp = cpool.tile([128, JW // 128, D], bf16, name="wp")
            # critical wave on the SP queue, finest first: stage B needs xT
            # cols 0:128 of each k-tile plus the head-0 Wq block first
            for k in range(KT):
                nc.sync.dma_start(
                    xT[:, k, 0:128], xT_d[k * 128 : (k + 1) * 128, 0:128]
                )
                nc.sync.dma_start(
                    wq[:, k, 0:D], wq_d[k * 128 : (k + 1) * 128, 0:D]
                )
            for k in range(KT):
                nc.sync.dma_start(
                    xT[:, k, 128:CW], xT_d[k * 128 : (k + 1) * 128, 128:CW]
                )
            for k in range(KT):
                nc.sync.dma_start(
                    xT[:, k, CW:N], xT_d[k * 128 : (k + 1) * 128, CW:N]
                )
            for k in range(KT):
                nc.sync.dma_start(xT8[:, k, :], xT8_d[k * 128 : (k + 1) * 128, :])
            # wq heads 1-3 (first needed ~110us in) ride the ScalarE hwdge
            # queue, which is idle until the first chunk's exps (~20us)
            for h in range(1, HPG):
                for k in range(KT):
                    nc.scalar.dma_start(
                        wq[:, k, h * D : (h + 1) * D],
                        wq_d[k * 128 : (k + 1) * 128, h * D : (h + 1) * D],
                    )

            # PE warmup: ~36 dummy matmuls on a memset tile while the input
            # DMAs land, so the HAM clock gate reaches 8/8 (2.4 GHz) before
            # stage B starts instead of ~25us into the kernel
            warm = cpool.tile([128, CW], bf16, name="warm")
            nc.vector.memset(warm[:, :], 1.0)
            ps_w = ps_stage.tile([128, CW], f32, name="ps_w", tag="stage")
            for _ in range(36):
                nc.tensor.matmul(
                    ps_w[:, :], lhsT=warm[:, 0:128], rhs=warm[:, :],
                    start=True, stop=True,
                )

            ones_col = cpool.tile([128, 1], bf16, name="ones_col")
            nc.vector.memset(ones_col[:, :], 1.0)
            # touch Exp once during the input-DMA wait so the ~2.7us ACT
            # table-set load is off the first chunk's critical path
            nc.scalar.activation(
                ones_col[:, :], ones_col[:, :],
                mybir.ActivationFunctionType.Exp, scale=0.0,
            )
            nc.vector.memset(ones_col[:, :], 1.0)
            # f32r inputs to the sum+broadcast matmul must be produced by
            # "rounding" writes, so stage through an f32 scratch tile
            f32r = mybir.dt.float32r
            ones128 = cpool.tile([128, 128], f32r, name="ones128")
            zpart = cpool.tile([128, CW], f32r, name="zpart")
            initt = y_pool.tile([128, D], f32, name="init", tag="y")
            nc.vector.memset(initt[:, :], 1.0)
            nc.vector.tensor_copy(ones128[:, :], initt[:, 0:128])
            nc.vector.memset(initt[:, :], 0.0)
            nc.vector.tensor_copy(zpart[:, :], initt[:, :])

            outTs = []

            # ---------- pipelined emission helpers ----------
            # pend: the chunk whose AV matmuls fill the current phase's
            # slots.  Keys: expS, qn, outT, rcpB, n0, ps (per-dt PSUM).
            def av_block(p, i):
                """Slot i of 16: 4 AV accumulation MMs for pending chunk p
                (dt = i//4, m-tiles 4*(i%4)..+4), plus the normalize mul
                when a dt completes."""
                if p is None:
                    return
                dt, m0 = i // 4, 4 * (i % 4)
                if m0 == 0:
                    p["ps"] = ps_av.tile([128, CW], f32, name="ps_av", tag="av")
                for mt in range(m0, m0 + 4):
                    nc.tensor.matmul(
                        p["ps"][:, :],
                        lhsT=p["qn"][:, mt, dt * 128 : (dt + 1) * 128],
                        rhs=p["expS"][:, mt, :],
                        start=(mt == 0),
                        stop=(mt == NT - 1),
                    )
                if m0 + 4 == NT:
                    nc.vector.tensor_mul(
                        p["outT"][:, dt, p["n0"] : p["n0"] + CW],
                        p["ps"][:, :], p["rcpB"][:, :],
                    )

            def den_start(p):
                """Denominator for the pending chunk, emitted at the top of
                the NEXT phase where all its exps are complete: 16 column-sum
                matmuls as 4 concurrent 4-packs in distinct PE column
                groups."""
                if p is None:
                    return
                psd = ps_den.tile([128, CW], f32, name="psd", tag="den")
                p["psd"] = psd
                for j in range(4):
                    for g in range(4):
                        nc.tensor.matmul(
                            psd[32 * g : 32 * g + 1, :],
                            lhsT=ones_col[:, :],
                            rhs=p["expS"][:, 4 * j + g, :],
                            start=(j == 0), stop=(j == 3),
                            tile_position=(0, 32 * g),
                        )

            def den_bcast(p):
                """Partial-row drains + K=4 sum/broadcast + reciprocal,
                emitted one slot after den_start so the DVE drains never
                stall the PE."""
                if p is None or "psd" not in p:
                    return
                psd = p["psd"]
                for g in range(4):
                    nc.vector.tensor_copy(
                        zpart[32 * g : 32 * g + 1, :], psd[32 * g : 32 * g + 1, :]
                    )
                psb = ps_den.tile([128, CW], f32, name="psb", tag="den")
                nc.tensor.matmul(
                    psb[:, :], lhsT=ones128[:, :], rhs=zpart[:, :],
                    start=True, stop=True,
                )
                nc.vector.reciprocal_approx_fast(p["rcpB"][:, :], psb[:, :])
                del p["psd"]

            def s_phase(h, c, qT8, qn, outT, pend):
                """Scores phase for chunk (h, c): 16 slots of [DR score
                pair + exp] interleaved with pend's AV; den groups for this
                chunk lag their exps by >=2 slots."""
                n0 = c * CW
                expS = exps_pool.tile([128, NT, CW], bf16, name="expS", tag="expS")
                den_start(pend)
                warm_ps = None
                if pend is None:
                    warm_ps = ps_stage.tile([128, CW], f32, name="ps_w2", tag="stage")
                cur = {
                    "expS": expS, "qn": qn, "outT": outT, "n0": n0,
                    "rcpB": rcp_pool.tile([128, CW], f32, name="rcpB", tag="rcpB"),
                }
                for mt in range(NT):
                    ps = ps_scores.tile([128, CW], f32, name="ps_s", tag="scores")
                    for t in range(KT // 2):
                        nc.tensor.matmul(
                            ps[:, :],
                            lhsT=xT8[:, 2 * t : 2 * t + 2, mt * 128 : (mt + 1) * 128],
                            rhs=qT8[:, 2 * t : 2 * t + 2, n0 : n0 + CW],
                            start=(t == 0),
                            stop=(t == KT // 2 - 1),
                            perf_mode=DR,
                        )
                    nc.scalar.activation(
                        expS[:, mt, :], ps[:, :],
                        mybir.ActivationFunctionType.Exp, scale=INV_SQRT_D,
                    )
                    av_block(pend, mt)
                    if pend is None and mt < 14:
                        # very first chunk: no AV fill exists yet, so the
                        # exp-paced gaps would re-throttle the HAM clock
                        # gate — keep the PE busy with a dummy matmul
                        nc.tensor.matmul(
                            warm_ps[:, :], lhsT=warm[:, 0:128], rhs=warm[:, :],
                            start=True, stop=True,
                        )
                    if mt == 0:
                        den_bcast(pend)
                return cur

            def b_phase(h, pend, first=False):
                """Stage B for head h: q_h = x Wq_h token-major (qn) plus
                qT8 via DMA xbar transpose slabs + DVE fp8 cast.  Runs bare
                (exp-independent, dense); the pending AV passes through to
                the following S(h,0) phase, which would otherwise be
                exp-paced.  Head 0 computes its first qT8 chunk on the PE
                (nothing earlier hides the transpose latency)."""
                j0 = h * D
                qT8 = qt8_pool.tile([128, KT, N], f8, name="qT8", tag="qT8")
                qn = qn_pool.tile([128, NT, D], bf16, name="qn", tag="qn")
                den_start(pend)

                def b_tile(mt):
                    ps = ps_stage.tile([128, D], f32, name="ps_b", tag="stage")
                    for k in range(KT):
                        nc.tensor.matmul(
                            ps[:, :],
                            lhsT=xT[:, k, mt * 128 : (mt + 1) * 128],
                            rhs=wq[:, k, j0 : j0 + D],
                            start=(k == 0),
                            stop=(k == KT - 1),
                        )
                    nc.vector.tensor_copy(qn[:, mt, :], ps[:, :])
                    if not (first and mt < 2 * (CW // 128)):
                        qts = qts_pool.tile(
                            [128, KT, 128], bf16, name="qts", tag="qts"
                        )
                        nc.sync.dma_start_transpose(qts[:, :, :], qn[:, mt, :])
                        nc.vector.tensor_copy(
                            qT8[:, :, mt * 128 : (mt + 1) * 128], qts[:, :, :]
                        )

                if first:
                    # chunks 0 and 1 of qT8 computed on the PE (transposed
                    # q via lhsT=Wq): the DMA transposes serialize at
                    # ~1.2us/dispatch behind the input loads, which would
                    # gate the first two scores phases
                    for mt in range(4):
                        b_tile(mt)
                        if mt == 0:
                            den_bcast(pend)
                    for jt in range(KT):
                        ps = ps_stage.tile([128, CW], f32, name="ps_a", tag="stage")
                        for k in range(KT):
                            nc.tensor.matmul(
                                ps[:, :],
                                lhsT=wq[:, k, jt * 128 : (jt + 1) * 128],
                                rhs=xT[:, k, 0:CW],
                                start=(k == 0),
                                stop=(k == KT - 1),
                            )
                        nc.vector.tensor_copy(qT8[:, jt, 0:CW], ps[:, :])
                    for mt in range(4, NT):
                        b_tile(mt)
                    for jt in range(KT):
                        ps = ps_stage.tile([128, CW], f32, name="ps_a", tag="stage")
                        for k in range(KT):
                            nc.tensor.matmul(
                                ps[:, :],
                                lhsT=wq[:, k, jt * 128 : (jt + 1) * 128],
                                rhs=xT[:, k, CW : 2 * CW],
                                start=(k == 0),
                                stop=(k == KT - 1),
                            )
                        nc.vector.tensor_copy(qT8[:, jt, CW : 2 * CW], ps[:, :])
                else:
                    for mt in range(NT):
                        b_tile(mt)
                        if mt == 0:
                            den_bcast(pend)
                return qT8, qn

            def proj_region(r):
                """Final projection for n-tiles 4r..4r+3 (requires every
                head's outT normalized for those columns)."""
                for nt in range(4 * r, 4 * r + 4):
                    ps = ps_stage.tile([128, D], f32, name="ps_y", tag="stage")
                    for hh in range(HPG):
                        for dt in range(KT):
                            jt = hh * KT + dt
                            nc.tensor.matmul(
                                ps[:, :],
                                lhsT=outTs[hh][:, dt, nt * 128 : (nt + 1) * 128],
                                rhs=wp[:, jt, :],
                                start=(jt == 0),
                                stop=(jt == HPG * KT - 1),
                            )
                    ysb = y_pool.tile([128, D], f32, name="ysb", tag="y")
                    if nt % 2 == 0:
                        nc.scalar.copy(ysb[:, :], ps[:, :])
                    else:
                        nc.vector.tensor_copy(ysb[:, :], ps[:, :])
                    nc.sync.dma_start(y_d[nt * 128 : (nt + 1) * 128, :], ysb[:, :])

            # ---------- the pipeline ----------
            pend = None
            for h in range(HPG):
                qT8, qn = b_phase(h, pend, first=(h == 0))
                if h == 0:
                    # wp (needed only by the projection) dispatches behind
                    # head 0's transposes on the SP queue
                    for j in range(JW // 128):
                        nc.sync.dma_start(
                            wp[:, j, :], wp_d[j * 128 : (j + 1) * 128, :]
                        )
                outT = outt_pool.tile(
                    [128, KT, N], bf16, name=f"outT{h}", tag=f"outT{h}"
                )
                outTs.append(outT)
                for c in range(NCHUNK):
                    cur = s_phase(h, c, qT8, qn, outT, pend)
                    if h == HPG - 1 and c >= 2:
                        proj_region(c - 2)
                    pend = cur
            # flush: last chunk's den + AV, then the two projection tail
            # regions
            den_start(pend)
            for i in range(NT):
                av_block(pend, i)
                if i == 0:
                    den_bcast(pend)
                if i == 7:
                    proj_region(2)
            proj_region(3)

    nc.compile()
    return nc


def _ensure_nc():
    if "nc" not in _state:
        _state["nc"] = _build()
    return _state["nc"]


def _make_in_maps(x, Wq, Wp):
    bf = np.float16
    f8 = ml_dtypes.float8_e4m3
    in_maps = []
    for c in range(NCORES):
        b, hg = c // HG, c % HG
        xt = np.ascontiguousarray(x[b].T)
        in_maps.append({
            "xt": xt.astype(bf),
            "xt8": xt.astype(f8),
            "wq": np.ascontiguousarray(Wq[:, hg * JW : (hg + 1) * JW]).astype(bf),
            "wp": np.ascontiguousarray(Wp[hg * JW : (hg + 1) * JW, :]).astype(bf),
        })
    return in_maps


def _get_runner():
    """Build once and cache a jitted 8-core runner (avoids re-jit per call)."""
    if "run" in _state:
        return _state["run"]

    import jax
    import concourse.mybir as mybir
    from jax.sharding import Mesh, PartitionSpec
    from jax.experimental.shard_map import shard_map
    from concourse import bass2jax

    nc = _ensure_nc()
    bass2jax.install_neuronx_cc_hook()

    partition_name = nc.partition_id_tensor.name if nc.partition_id_tensor else None
    in_names, out_names, out_avals, zero_outs = [], [], [], []
    for alloc in nc.m.functions[0].allocations:
        if not isinstance(alloc, mybir.MemoryLocationSet):
            continue
        name = alloc.memorylocations[0].name
        if alloc.kind == "ExternalInput":
            if name != partition_name:
                in_names.append(name)
        elif alloc.kind == "ExternalOutput":
            shape = tuple(alloc.tensor_shape)
            dtype = mybir.dt.np(alloc.dtype)
            out_avals.append(jax.core.ShapedArray(shape, dtype))
            out_names.append(name)
            zero_outs.append(np.zeros(shape, dtype))
    n_params = len(in_names)
    n_outs = len(out_names)
    all_in_names = list(in_names) + list(out_names)
    if partition_name is not None:
        all_in_names.append(partition_name)

    def _body(*args):
        operands = list(args)
        if partition_name is not None:
            operands.append(bass2jax.partition_id_tensor())
        outs = bass2jax._bass_exec_p.bind(
            *operands,
            out_avals=tuple(out_avals),
            in_names=tuple(all_in_names),
            out_names=tuple(out_names),
            lowering_input_output_aliases=(),
            sim_require_finite=True,
            sim_require_nnan=True,
            nc=nc,
        )
        return tuple(outs)

    devices = jax.devices()[:NCORES]
    mesh = Mesh(np.asarray(devices), ("core",))
    in_specs = (PartitionSpec("core"),) * (n_params + n_outs)
    out_specs = (PartitionSpec("core"),) * n_outs
    sharded = jax.jit(
        shard_map(_body, mesh=mesh, in_specs=in_specs, out_specs=out_specs,
                  check_rep=False),
        donate_argnums=tuple(range(n_params, n_params + n_outs)),
        keep_unused=True,
    )

    def run(in_maps):
        concat_in = [
            np.concatenate([np.asarray(m[name]) for m in in_maps], axis=0)
            for name in in_names
        ]
        concat_zeros = [
            np.zeros((NCORES * z.shape[0], *z.shape[1:]), z.dtype) for z in zero_outs
        ]
        out_arrs = sharded(*concat_in, *concat_zeros)
        return [
            {
                name: np.asarray(out_arrs[i]).reshape(NCORES, *out_avals[i].shape)[c]
                for i, name in enumerate(out_names)
            }
            for c in range(NCORES)
        ]

    _state["run"] = run
    return run


def kernel(x, Wq, Wv, Wp, bp):
    x = np.asarray(x, np.float32)
    Wq = np.asarray(Wq, np.float32)
    Wp = np.asarray(Wp, np.float32)
    bp = np.asarray(bp, np.float32)

    run = _get_runner()
    results = run(_make_in_maps(x, Wq, Wp))
    y = np.empty((B, N, D), np.float32)
    for b in range(B):
        y[b] = results[b * HG]["y"] + results[b * HG + 1]["y"] + bp[None, :]
    return y



# revision 2
# speedup vs baseline: 1.0816x; 1.0816x over previous
"""Multi-head attention V2 kernel for Trainium2 (8 NeuronCores).

Problem shapes (hardcoded): x [4, 2048, 512] f32, Wq [512, 4096], Wv unused,
Wp [4096, 512], bp [512].  Reference math (note: V uses the Q projection):
    q = v = (x @ Wq) -> [B, H, N, D] with H=8, head dim = D = 512
    S = q @ x^T / sqrt(D);  P = softmax(S, -1);  out = (P @ v) @ Wp + bp

Sharding: core = (batch b, head-group hg) with 2 groups of 4 heads.

Weight folding (host): M_h = Wq_h @ Wp_h [D, D] per head, so
    y = sum_h P_h @ (x @ M_h) + bp
which eliminates the device-side output projection: per-head AV matmuls
use xM_h = x @ M_h as the value operand and their PSUM results merge
(scaled by the softmax reciprocal) into a single y^T accumulator in SBUF
on the DVE.  The xM_h tiles are computed on the PE in a prologue that
replaces the old dummy-matmul HAM warmup with real work.

q^T (fp8, scores rhs) is computed directly on the PE as Wq_h^T x^T
(lhsT = wq) and cast f32->fp8 by the DVE -- no token-major q, no DMA
xbar transposes, no SP-queue pressure, no head-boundary cast stalls.

The scores matmul S^T = x q^T runs in fp8e4 DoubleRow mode (K=256 per
instruction); both operands are TRN fp8_e4m3 (xT8 host-converted, qT8
cast on-device from f32 PSUM).  Everything else is fp16 with fp32 PSUM
accumulation; the y^T accumulator and output are f32.

Pipelining: identical slot discipline to V1 -- each s_phase interleaves
chunk c's 16 [DR score pair + exp] slots with the pending chunk's 64 AV
matmuls (1 pair : 4 AV MMs per slot, measured slot 1310ns), denominator
matmuls (ones^T expS, 4-way column-group packs) run at the top of the
following phase, lagging their exps.  qt_phases (64 MMs, exp-free) run
bare between heads; s_phase(0,0), which has no pending AV, interleaves
the last head's xM tiles as filler instead of dummy matmuls.  Head 3's
AV merge DMAs each completed y^T chunk straight out.
"""

import sys

sys.path.insert(0, "/opt/trn_rl_repo")

import numpy as np
import ml_dtypes

B, N, D, H = 4, 2048, 512, 8
NCORES = 8
HG = 2            # head groups (cores per batch)
HPG = H // HG     # heads per core
JW = HPG * D      # per-core Wq column count (2048)
KT = D // 128     # k-tiles over feature dim (4)
NT = N // 128     # partition tiles over tokens (16)
NCHUNK = 4        # n split into 4 chunks of 512
CW = N // NCHUNK  # chunk width (512)
INV_SQRT_D = 1.0 / float(np.sqrt(D))

_state = {}


def _build():
    import concourse.bass as bass
    import concourse.mybir as mybir
    import concourse.tile as tile
    from concourse import bacc

    f32 = mybir.dt.float32
    bf16 = mybir.dt.float16
    f8 = mybir.dt.float8e4
    DR = mybir.MatmulPerfMode.DoubleRow

    nc = bacc.Bacc("TRN2", target_bir_lowering=False)

    xT_d = nc.dram_tensor("xt", [D, N], bf16, kind="ExternalInput")
    xT8_d = nc.dram_tensor("xt8", [D, N], f8, kind="ExternalInput")
    wq_d = nc.dram_tensor("wq", [D, JW], bf16, kind="ExternalInput")
    wm_d = nc.dram_tensor("wm", [D, JW], bf16, kind="ExternalInput")
    yT_d = nc.dram_tensor("yt", [D, N], f32, kind="ExternalOutput")

    with tile.TileContext(nc) as tc:
        with (
            tc.tile_pool(name="const", bufs=1) as cpool,
            tc.tile_pool(name="qt8", bufs=1) as qt8_pool,
            tc.tile_pool(name="exps", bufs=2) as exps_pool,
            tc.tile_pool(name="rcp", bufs=2) as rcp_pool,
            tc.tile_pool(name="mrg", bufs=2) as mrg_pool,
            tc.tile_pool(name="ps_stage", bufs=2, space="PSUM") as ps_stage,
            tc.tile_pool(name="ps_scores", bufs=3, space="PSUM") as ps_scores,
            tc.tile_pool(name="ps_av", bufs=2, space="PSUM") as ps_av,
            tc.tile_pool(name="ps_den", bufs=1, space="PSUM") as ps_den,
        ):
            # ---- resident inputs ----
            xT = cpool.tile([128, KT, N], bf16, name="xT")
            xT8 = cpool.tile([128, KT, N], f8, name="xT8")
            wq = cpool.tile([128, KT, JW], bf16, name="wq")
            wm = cpool.tile([128, KT, JW], bf16, name="wm")
            # critical wave, finest first: the xM prologue consumes wm
            # head 0 plus xT column blocks in mt order
            for k in range(KT):
                nc.sync.dma_start(wm[:, k, 0:D], wm_d[k * 128 : (k + 1) * 128, 0:D])
            for cn in range(NCHUNK):
                for k in range(KT):
                    nc.sync.dma_start(
                        xT[:, k, cn * CW : (cn + 1) * CW],
                        xT_d[k * 128 : (k + 1) * 128, cn * CW : (cn + 1) * CW],
                    )
            for h in range(1, HPG - 1):
                for k in range(KT):
                    nc.sync.dma_start(
                        wm[:, k, h * D : (h + 1) * D],
                        wm_d[k * 128 : (k + 1) * 128, h * D : (h + 1) * D],
                    )
            # wq needed at qt_phase(0) (~48us); xT8 at s_phase(0,0) (~62us)
            for k in range(KT):
                nc.sync.dma_start(wq[:, k, :], wq_d[k * 128 : (k + 1) * 128, :])
            for k in range(KT):
                nc.sync.dma_start(xT8[:, k, :], xT8_d[k * 128 : (k + 1) * 128, :])
            # wm head 3 (needed ~82us) rides the idle ScalarE hwdge queue
            for k in range(KT):
                nc.scalar.dma_start(
                    wm[:, k, (HPG - 1) * D : HPG * D],
                    wm_d[k * 128 : (k + 1) * 128, (HPG - 1) * D : HPG * D],
                )

            ones_col = cpool.tile([128, 1], bf16, name="ones_col")
            nc.vector.memset(ones_col[:, :], 1.0)
            # touch Exp once during the input-DMA wait so the ~2.7us ACT
            # table-set load is off the first chunk's critical path
            nc.scalar.activation(
                ones_col[:, :], ones_col[:, :],
                mybir.ActivationFunctionType.Exp, scale=0.0,
            )
            nc.vector.memset(ones_col[:, :], 1.0)
            # f32r inputs to the sum+broadcast matmul must be produced by
            # "rounding" writes, so stage through an f32 scratch tile
            f32r = mybir.dt.float32r
            ones128 = cpool.tile([128, 128], f32r, name="ones128")
            zpart = cpool.tile([128, CW], f32r, name="zpart")
            initt = mrg_pool.tile([128, CW], f32, name="init", tag="mrg")
            nc.vector.memset(initt[:, :], 1.0)
            nc.vector.tensor_copy(ones128[:, :], initt[:, 0:128])
            nc.vector.memset(initt[:, :], 0.0)
            nc.vector.tensor_copy(zpart[:, :], initt[:, :])

            # per-head xM tiles (token-major values) and per-chunk y^T
            # accumulators
            xmn = [
                cpool.tile([128, NT, D], bf16, name=f"xmn{h}") for h in range(HPG)
            ]
            ysb = [
                cpool.tile([128, KT, CW], f32, name=f"ysb{c}") for c in range(NCHUNK)
            ]

            # ---------- pipelined emission helpers ----------
            def xmn_block(h, mt):
                """xM_h token-major tile mt: 4 accumulation MMs + DVE cast."""
                ps = ps_stage.tile([128, D], f32, name="ps_x", tag="stage")
                for k in range(KT):
                    nc.tensor.matmul(
                        ps[:, :],
                        lhsT=xT[:, k, mt * 128 : (mt + 1) * 128],
                        rhs=wm[:, k, h * D : (h + 1) * D],
                        start=(k == 0),
                        stop=(k == KT - 1),
                    )
                nc.vector.tensor_copy(xmn[h][:, mt, :], ps[:, :])

            def av_block(p, i):
                """Slot i of 16: 4 AV accumulation MMs for pending chunk p
                (dt = i//4, m-tiles 4*(i%4)..+4); when a dt completes, merge
                the scaled result into the y^T accumulator (and DMA it out
                on the last head)."""
                if p is None:
                    return
                dt, m0 = i // 4, 4 * (i % 4)
                if m0 == 0:
                    p["ps"] = ps_av.tile([128, CW], f32, name="ps_av", tag="av")
                for mt in range(m0, m0 + 4):
                    nc.tensor.matmul(
                        p["ps"][:, :],
                        lhsT=p["xmn"][:, mt, dt * 128 : (dt + 1) * 128],
                        rhs=p["expS"][:, mt, :],
                        start=(mt == 0),
                        stop=(mt == NT - 1),
                    )
                if m0 + 4 == NT:
                    dst = ysb[p["c"]][:, dt, :]
                    if p["h"] == 0:
                        nc.vector.tensor_mul(dst, p["ps"][:, :], p["rcpB"][:, :])
                    else:
                        t = mrg_pool.tile([128, CW], f32, name="mrg", tag="mrg")
                        nc.vector.tensor_mul(t[:, :], p["ps"][:, :], p["rcpB"][:, :])
                        nc.vector.tensor_add(dst, dst, t[:, :])
                    if p["h"] == HPG - 1:
                        nc.sync.dma_start(
                            yT_d[dt * 128 : (dt + 1) * 128,
                                 p["n0"] : p["n0"] + CW],
                            dst,
                        )

            def den_start(p):
                """Denominator for the pending chunk, emitted at the top of
                the NEXT phase where all its exps are complete: 16 column-sum
                matmuls as 4 concurrent 4-packs in distinct PE column
                groups."""
                if p is None or "den" in p:
                    return
                p["den"] = True
                psd = ps_den.tile([128, CW], f32, name="psd", tag="den")
                p["psd"] = psd
                for j in range(4):
                    for g in range(4):
                        nc.tensor.matmul(
                            psd[32 * g : 32 * g + 1, :],
                            lhsT=ones_col[:, :],
                            rhs=p["expS"][:, 4 * j + g, :],
                            start=(j == 0), stop=(j == 3),
                            tile_position=(0, 32 * g),
                        )

            def den_bcast(p):
                """Partial-row drains + K=4 sum/broadcast + reciprocal,
                emitted one slot after den_start so the DVE drains never
                stall the PE."""
                if p is None or "psd" not in p:
                    return
                psd = p["psd"]
                for g in range(4):
                    nc.vector.tensor_copy(
                        zpart[32 * g : 32 * g + 1, :], psd[32 * g : 32 * g + 1, :]
                    )
                psb = ps_den.tile([128, CW], f32, name="psb", tag="den")
                nc.tensor.matmul(
                    psb[:, :], lhsT=ones128[:, :], rhs=zpart[:, :],
                    start=True, stop=True,
                )
                nc.vector.reciprocal_approx_fast(p["rcpB"][:, :], psb[:, :])
                del p["psd"]

            def qt_phase(h, pend):
                """q^T (fp8) for head h, computed directly on the PE as
                Wq_h^T x^T: 16 stages of 4 accumulation MMs + f32->fp8 DVE
                cast.  Runs bare (exp-independent, dense); the pending AV
                passes through to the following s_phase(h,0).  cn-major
                order so chunk 0's scores operands land first."""
                qT8 = qt8_pool.tile([128, KT, N], f8, name="qT8", tag="qT8")
                den_start(pend)
                first = True
                for cn in range(NCHUNK):
                    for jb in range(KT):
                        ps = ps_stage.tile([128, CW], f32, name="ps_q", tag="stage")
                        for k in range(KT):
                            nc.tensor.matmul(
                                ps[:, :],
                                lhsT=wq[:, k, h * D + jb * 128 : h * D + (jb + 1) * 128],
                                rhs=xT[:, k, cn * CW : (cn + 1) * CW],
                                start=(k == 0),
                                stop=(k == KT - 1),
                            )
                        nc.vector.tensor_copy(qT8[:, jb, cn * CW : (cn + 1) * CW], ps[:, :])
                        if first:
                            den_bcast(pend)
                            first = False
                return qT8

            def s_phase(h, c, qT8, pend, filler=None):
                """Scores phase for chunk (h, c): 16 slots of [DR score
                pair + exp] interleaved with pend's AV; den groups for this
                chunk lag their exps by >=2 slots.  filler (s_phase(0,0)
                only, where no pending AV exists) emits real PE work to
                keep the slot structure dense."""
                n0 = c * CW
                expS = exps_pool.tile([128, NT, CW], bf16, name="expS", tag="expS")
                den_start(pend)
                cur = {
                    "expS": expS, "xmn": xmn[h], "h": h, "c": c, "n0": n0,
                    "rcpB": rcp_pool.tile([128, CW], f32, name="rcpB", tag="rcpB"),
                }
                for mt in range(NT):
                    ps = ps_scores.tile([128, CW], f32, name="ps_s", tag="scores")
                    for t in range(KT // 2):
                        nc.tensor.matmul(
                            ps[:, :],
                            lhsT=xT8[:, 2 * t : 2 * t + 2, mt * 128 : (mt + 1) * 128],
                            rhs=qT8[:, 2 * t : 2 * t + 2, n0 : n0 + CW],
                            start=(t == 0),
                            stop=(t == KT // 2 - 1),
                            perf_mode=DR,
                        )
                    nc.scalar.activation(
                        expS[:, mt, :], ps[:, :],
                        mybir.ActivationFunctionType.Exp, scale=INV_SQRT_D,
                    )
                    av_block(pend, mt)
                    if filler is not None:
                        filler(mt)
                    if mt == 0:
                        den_bcast(pend)
                return cur

            # ---------- the pipeline ----------
            # prologue: xM for heads 0..2 doubles as the HAM warmup (real
            # work from the first instruction); head 3's xM is s_phase(0,0)
            # filler
            for h in range(HPG - 1):
                for mt in range(NT):
                    xmn_block(h, mt)
            pend = None
            for h in range(HPG):
                qT8 = qt_phase(h, pend)
                for c in range(NCHUNK):
                    filler = None
                    if h == 0 and c == 0:
                        filler = lambda mt: xmn_block(HPG - 1, mt)
                    cur = s_phase(h, c, qT8, pend, filler)
                    pend = cur
            # flush: last chunk's den + AV (merge DMAs the final y^T chunk)
            den_start(pend)
            for i in range(NT):
                av_block(pend, i)
                if i == 0:
                    den_bcast(pend)

    nc.compile()
    return nc


def _ensure_nc():
    if "nc" not in _state:
        _state["nc"] = _build()
    return _state["nc"]


def _make_in_maps(x, Wq, Wp):
    bf = np.float16
    f8 = ml_dtypes.float8_e4m3
    # fold the output projection into per-head value matrices:
    # M_h = Wq_h @ Wp_h  (weight-only, input-independent)
    wms = []
    for hg in range(HG):
        Mi = np.empty((D, JW), np.float32)
        for hh in range(HPG):
            g = hg * HPG + hh
            Mi[:, hh * D : (hh + 1) * D] = (
                Wq[:, g * D : (g + 1) * D] @ Wp[g * D : (g + 1) * D, :]
            )
        wms.append(Mi.astype(bf))
    in_maps = []
    for c in range(NCORES):
        b, hg = c // HG, c % HG
        xt = np.ascontiguousarray(x[b].T)
        in_maps.append({
            "xt": xt.astype(bf),
            "xt8": xt.astype(f8),
            "wq": np.ascontiguousarray(Wq[:, hg * JW : (hg + 1) * JW]).astype(bf),
            "wm": wms[hg],
        })
    return in_maps


def _get_runner():
    """Build once and cache a jitted 8-core runner (avoids re-jit per call)."""
    if "run" in _state:
        return _state["run"]

    import jax
    import concourse.mybir as mybir
    from jax.sharding import Mesh, PartitionSpec
    from jax.experimental.shard_map import shard_map
    from concourse import bass2jax

    nc = _ensure_nc()
    bass2jax.install_neuronx_cc_hook()

    partition_name = nc.partition_id_tensor.name if nc.partition_id_tensor else None
    in_names, out_names, out_avals, zero_outs = [], [], [], []
    for alloc in nc.m.functions[0].allocations:
        if not isinstance(alloc, mybir.MemoryLocationSet):
            continue
        name = alloc.memorylocations[0].name
        if alloc.kind == "ExternalInput":
            if name != partition_name:
                in_names.append(name)
        elif alloc.kind == "ExternalOutput":
            shape = tuple(alloc.tensor_shape)
            dtype = mybir.dt.np(alloc.dtype)
            out_avals.append(jax.core.ShapedArray(shape, dtype))
            out_names.append(name)
            zero_outs.append(np.zeros(shape, dtype))
    n_params = len(in_names)
    n_outs = len(out_names)
    all_in_names = list(in_names) + list(out_names)
    if partition_name is not None:
        all_in_names.append(partition_name)

    def _body(*args):
        operands = list(args)
        if partition_name is not None:
            operands.append(bass2jax.partition_id_tensor())
        outs = bass2jax._bass_exec_p.bind(
            *operands,
            out_avals=tuple(out_avals),
            in_names=tuple(all_in_names),
            out_names=tuple(out_names),
            lowering_input_output_aliases=(),
            sim_require_finite=True,
            sim_require_nnan=True,
            nc=nc,
        )
        return tuple(outs)

    devices = jax.devices()[:NCORES]
    mesh = Mesh(np.asarray(devices), ("core",))
    in_specs = (PartitionSpec("core"),) * (n_params + n_outs)
    out_specs = (PartitionSpec("core"),) * n_outs
    sharded = jax.jit(
        shard_map(_body, mesh=mesh, in_specs=in_specs, out_specs=out_specs,
                  check_rep=False),
        donate_argnums=tuple(range(n_params, n_params + n_outs)),
        keep_unused=True,
    )

    def run(in_maps):
        concat_in = [
            np.concatenate([np.asarray(m[name]) for m in in_maps], axis=0)
            for name in in_names
        ]
        concat_zeros = [
            np.zeros((NCORES * z.shape[0], *z.shape[1:]), z.dtype) for z in zero_outs
        ]
        out_arrs = sharded(*concat_in, *concat_zeros)
        return [
            {
                name: np.asarray(out_arrs[i]).reshape(NCORES, *out_avals[i].shape)[c]
                for i, name in enumerate(out_names)
            }
            for c in range(NCORES)
        ]

    _state["run"] = run
    return run


def kernel(x, Wq, Wv, Wp, bp):
    x = np.asarray(x, np.float32)
    Wq = np.asarray(Wq, np.float32)
    Wp = np.asarray(Wp, np.float32)
    bp = np.asarray(bp, np.float32)

    run = _get_runner()
    results = run(_make_in_maps(x, Wq, Wp))
    y = np.empty((B, N, D), np.float32)
    for b in range(B):
        yt = results[b * HG]["yt"] + results[b * HG + 1]["yt"]
        y[b] = yt.T + bp[None, :]
    return y


# revision 12
# speedup vs baseline: 1.1136x; 1.0296x over previous
"""Multi-head attention V2 kernel for Trainium2 (8 NeuronCores).

Problem shapes (hardcoded): x [4, 2048, 512] f32, Wq [512, 4096], Wv unused,
Wp [4096, 512], bp [512].  Reference math (note: V uses the Q projection):
    q = v = (x @ Wq) -> [B, H, N, D] with H=8, head dim = D = 512
    S = q @ x^T / sqrt(D);  P = softmax(S, -1);  out = (P @ v) @ Wp + bp

Sharding: core = (batch b, head-group hg) with 2 groups of 4 heads.

Weight folding (host): M_h = Wq_h @ Wp_h [D, D] per head, so
    y = sum_h P_h @ (x @ M_h) + bp
which eliminates the device-side output projection: per-head AV matmuls
use xM_h = x @ M_h as the value operand and their PSUM results merge
(scaled by the softmax reciprocal) into a single y^T accumulator in SBUF
on the DVE.  The xM_h tiles are computed on the PE in a prologue that
replaces the old dummy-matmul HAM warmup with real work.

q^T (fp8, scores rhs) is computed directly on the PE as Wq_h^T x^T
(lhsT = wq) and cast f32->fp8 by the DVE -- no token-major q, no DMA
xbar transposes, no SP-queue pressure, no head-boundary cast stalls.

The scores matmul S^T = x q^T runs in fp8e4 DoubleRow mode (K=256 per
instruction); both operands are TRN fp8_e4m3 (xT8 host-converted, qT8
cast on-device from f32 PSUM).  Everything else is fp16 with fp32 PSUM
accumulation; the y^T accumulator and output are f32.

Pipelining: identical slot discipline to V1 -- each s_phase interleaves
chunk c's 16 [DR score pair + exp] slots with the pending chunk's 64 AV
matmuls (1 pair : 4 AV MMs per slot, measured slot 1310ns), denominator
matmuls (ones^T expS, 4-way column-group packs) run at the top of the
following phase, lagging their exps.  qt_phases (64 MMs, exp-free) run
bare between heads; s_phase(0,0), which has no pending AV, interleaves
the last head's xM tiles as filler instead of dummy matmuls.  Head 3's
AV merge DMAs each completed y^T chunk straight out.
"""

import sys

sys.path.insert(0, "/opt/trn_rl_repo")

import numpy as np
import ml_dtypes

B, N, D, H = 4, 2048, 512, 8
NCORES = 8
HG = 2            # head groups (cores per batch)
HPG = H // HG     # heads per core
JW = HPG * D      # per-core Wq column count (2048)
KT = D // 128     # k-tiles over feature dim (4)
NT = N // 128     # partition tiles over tokens (16)
NCHUNK = 4        # n split into 4 chunks of 512
CW = N // NCHUNK  # chunk width (512)
INV_SQRT_D = 1.0 / float(np.sqrt(D))

_state = {}


def _build():
    import concourse.bass as bass
    import concourse.mybir as mybir
    import concourse.tile as tile
    from concourse import bacc

    f32 = mybir.dt.float32
    bf16 = mybir.dt.float16
    f8 = mybir.dt.float8e4
    DR = mybir.MatmulPerfMode.DoubleRow

    nc = bacc.Bacc("TRN2", target_bir_lowering=False)

    xT_d = nc.dram_tensor("xt", [D, N], bf16, kind="ExternalInput")
    xT8_d = nc.dram_tensor("xt8", [D, N], f8, kind="ExternalInput")
    wq_d = nc.dram_tensor("wq", [D, JW], bf16, kind="ExternalInput")
    wm_d = nc.dram_tensor("wm", [D, JW], bf16, kind="ExternalInput")
    yT_d = nc.dram_tensor("yt", [D, N], f32, kind="ExternalOutput")

    with tile.TileContext(nc) as tc:
        with (
            tc.tile_pool(name="const", bufs=1) as cpool,
            tc.tile_pool(name="qt8", bufs=1) as qt8_pool,
            tc.tile_pool(name="exps", bufs=2) as exps_pool,
            tc.tile_pool(name="rcp", bufs=2) as rcp_pool,
            tc.tile_pool(name="mrg", bufs=2) as mrg_pool,
            tc.tile_pool(name="acc", bufs=2) as acc_pool,
            tc.tile_pool(name="zp", bufs=1) as zp_pool,
            tc.tile_pool(name="ps_stage", bufs=2, space="PSUM") as ps_stage,
            tc.tile_pool(name="ps_scores", bufs=3, space="PSUM") as ps_scores,
            tc.tile_pool(name="ps_av", bufs=2, space="PSUM") as ps_av,
            tc.tile_pool(name="ps_den", bufs=1, space="PSUM") as ps_den,
        ):
            # ---- resident inputs ----
            xT = cpool.tile([128, KT, N], bf16, name="xT")
            xT8 = cpool.tile([128, KT, N], f8, name="xT8")
            wq = cpool.tile([128, KT, JW], bf16, name="wq")
            wm = cpool.tile([128, KT, JW], bf16, name="wm")
            # critical wave spread over four hwdge queues so the first
            # xM tile's operands (xT cn0 + wm head 0) land ~4us in:
            #   sync:   xT (cn-major, mt order the prologue consumes)
            #   scalar: wm heads 0-1, then wq (needed at qt_phase(0), ~42us)
            #   gpsimd: wm heads 2-3 (~28us/~75us), then xT8 (~56us)
            for cn in range(NCHUNK):
                for k in range(KT):
                    nc.sync.dma_start(
                        xT[:, k, cn * CW : (cn + 1) * CW],
                        xT_d[k * 128 : (k + 1) * 128, cn * CW : (cn + 1) * CW],
                    )
            for h in range(2):
                for k in range(KT):
                    nc.scalar.dma_start(
                        wm[:, k, h * D : (h + 1) * D],
                        wm_d[k * 128 : (k + 1) * 128, h * D : (h + 1) * D],
                    )
            for k in range(KT):
                nc.scalar.dma_start(wq[:, k, :], wq_d[k * 128 : (k + 1) * 128, :])
            for h in range(2, HPG):
                for k in range(KT):
                    nc.gpsimd.dma_start(
                        wm[:, k, h * D : (h + 1) * D],
                        wm_d[k * 128 : (k + 1) * 128, h * D : (h + 1) * D],
                    )
            for k in range(KT):
                nc.gpsimd.dma_start(xT8[:, k, :], xT8_d[k * 128 : (k + 1) * 128, :])

            ones_col = cpool.tile([128, 1], bf16, name="ones_col")
            nc.vector.memset(ones_col[:, :], 1.0)
            # touch Exp once during the input-DMA wait so the ~2.7us ACT
            # table-set load is off the first chunk's critical path
            nc.scalar.activation(
                ones_col[:, :], ones_col[:, :],
                mybir.ActivationFunctionType.Exp, scale=0.0,
            )
            # f32r inputs to the sum+broadcast matmul must be produced by
            # "rounding" writes, so stage through a scratch tile
            f32r = mybir.dt.float32r
            ones128 = cpool.tile([128, 128], f32r, name="ones128")
            scr = acc_pool.tile([128, CW], bf16, name="scr", tag="acc")
            nc.vector.memset(scr[:, :], 1.0)
            nc.vector.tensor_copy(ones128[:, :], scr[:, 0:128])

            # per-head xM tiles (token-major values) and per-chunk y^T
            # accumulators
            xmn = [
                cpool.tile([128, NT, D], bf16, name=f"xmn{h}") for h in range(HPG)
            ]
            ysb = [
                cpool.tile([128, KT, CW], f32, name=f"ysb{c}") for c in range(NCHUNK)
            ]

            # ---------- pipelined emission helpers ----------
            def xmn_block(h, mt):
                """xM_h token-major tile mt: 4 accumulation MMs + DVE cast."""
                ps = ps_stage.tile([128, D], f32, name="ps_x", tag="stage")
                for k in range(KT):
                    nc.tensor.matmul(
                        ps[:, :],
                        lhsT=xT[:, k, mt * 128 : (mt + 1) * 128],
                        rhs=wm[:, k, h * D : (h + 1) * D],
                        start=(k == 0),
                        stop=(k == KT - 1),
                    )
                nc.vector.tensor_copy(xmn[h][:, mt, :], ps[:, :])

            def av_block(p, i):
                """Slot i of 16: 4 AV accumulation MMs for pending chunk p
                (dt = i//4, m-tiles 4*(i%4)..+4); when a dt completes, merge
                the scaled result into the y^T accumulator (and DMA it out
                on the last head)."""
                if p is None:
                    return
                dt, m0 = i // 4, 4 * (i % 4)
                if m0 == 0:
                    p["ps"] = ps_av.tile([128, CW], f32, name="ps_av", tag="av")
                for mt in range(m0, m0 + 4):
                    nc.tensor.matmul(
                        p["ps"][:, :],
                        lhsT=p["xmn"][:, mt, dt * 128 : (dt + 1) * 128],
                        rhs=p["expS"][:, mt, :],
                        start=(mt == 0),
                        stop=(mt == NT - 1),
                    )
                if m0 + 4 == NT:
                    dst = ysb[p["c"]][:, dt, :]
                    if p["h"] == 0:
                        nc.vector.tensor_mul(dst, p["ps"][:, :], p["rcpB"][:, :])
                    else:
                        t = mrg_pool.tile([128, CW], bf16, name="mrg", tag="mrg")
                        nc.vector.tensor_mul(t[:, :], p["ps"][:, :], p["rcpB"][:, :])
                        nc.vector.tensor_add(dst, dst, t[:, :])
                    if p["h"] == HPG - 1:
                        nc.sync.dma_start(
                            yT_d[dt * 128 : (dt + 1) * 128,
                                 p["n0"] : p["n0"] + CW],
                            dst,
                        )

            def den_bcast(p):
                """Denominator finish for the pending chunk, emitted one
                slot into the NEXT phase: the 16 expS m-tiles were summed
                elementwise by a DVE running chain during p's own phase
                (p["zp"], f32r); one K=128 sum+broadcast matmul and the
                reciprocal produce rcpB."""
                if p is None or "zp" not in p:
                    return
                psb = ps_den.tile([128, CW], f32, name="psb", tag="den")
                nc.tensor.matmul(
                    psb[:, :], lhsT=ones128[:, :], rhs=p["zp"][:, :],
                    start=True, stop=True,
                )
                nc.vector.reciprocal_approx_fast(p["rcpB"][:, :], psb[:, :])
                del p["zp"]

            def qt_phase(h, pend):
                """q^T (fp8) for head h, computed directly on the PE as
                Wq_h^T x^T: 16 stages of 4 accumulation MMs + f32->fp8 DVE
                cast.  Runs bare (exp-independent, dense); the pending AV
                passes through to the following s_phase(h,0).  cn-major
                order so chunk 0's scores operands land first."""
                qT8 = qt8_pool.tile([128, KT, N], f8, name="qT8", tag="qT8")
                first = True
                for cn in range(NCHUNK):
                    for jb in range(KT):
                        ps = ps_stage.tile([128, CW], f32, name="ps_q", tag="stage")
                        for k in range(KT):
                            nc.tensor.matmul(
                                ps[:, :],
                                lhsT=wq[:, k, h * D + jb * 128 : h * D + (jb + 1) * 128],
                                rhs=xT[:, k, cn * CW : (cn + 1) * CW],
                                start=(k == 0),
                                stop=(k == KT - 1),
                            )
                        nc.vector.tensor_copy(qT8[:, jb, cn * CW : (cn + 1) * CW], ps[:, :])
                        if first:
                            den_bcast(pend)
                            first = False
                return qT8

            def s_phase(h, c, qT8, pend, filler=None):
                """Scores phase for chunk (h, c): 16 slots of [DR score
                pair + exp] interleaved with pend's AV; den groups for this
                chunk lag their exps by >=2 slots.  filler (s_phase(0,0)
                only, where no pending AV exists) emits real PE work to
                keep the slot structure dense."""
                n0 = c * CW
                expS = exps_pool.tile([128, NT, CW], bf16, name="expS", tag="expS")
                cur = {
                    "expS": expS, "xmn": xmn[h], "h": h, "c": c, "n0": n0,
                    "rcpB": rcp_pool.tile([128, CW], f32, name="rcpB", tag="rcpB"),
                }
                for mt in range(NT):
                    ps = ps_scores.tile([128, CW], f32, name="ps_s", tag="scores")
                    for t in range(KT // 2):
                        nc.tensor.matmul(
                            ps[:, :],
                            lhsT=xT8[:, 2 * t : 2 * t + 2, mt * 128 : (mt + 1) * 128],
                            rhs=qT8[:, 2 * t : 2 * t + 2, n0 : n0 + CW],
                            start=(t == 0),
                            stop=(t == KT // 2 - 1),
                            perf_mode=DR,
                        )
                    nc.scalar.activation(
                        expS[:, mt, :], ps[:, :],
                        mybir.ActivationFunctionType.Exp, scale=INV_SQRT_D,
                    )
                    # running elementwise sum of the expS m-tiles (DVE):
                    # replaces the packed PE column-sum matmuls
                    if mt == 1:
                        acc = acc_pool.tile([128, CW], bf16, name="acc", tag="acc")
                        cur["acc"] = acc
                        nc.vector.tensor_add(
                            acc[:, :], expS[:, 0, :], expS[:, 1, :]
                        )
                    elif mt >= 2:
                        acc = cur["acc"]
                        nc.vector.tensor_add(acc[:, :], acc[:, :], expS[:, mt, :])
                    av_block(pend, mt)
                    if filler is not None:
                        filler(mt)
                    if mt == 0:
                        den_bcast(pend)
                zp = zp_pool.tile([128, CW], mybir.dt.float32r, name="zp", tag="zp")
                nc.vector.tensor_copy(zp[:, :], cur["acc"][:, :])
                cur["zp"] = zp
                return cur

            # ---------- the pipeline ----------
            # prologue: xM for heads 0..2 doubles as the HAM warmup (real
            # work from the first instruction); head 3's xM is s_phase(0,0)
            # filler
            for h in range(HPG - 1):
                for mt in range(NT):
                    xmn_block(h, mt)
            pend = None
            for h in range(HPG):
                qT8 = qt_phase(h, pend)
                for c in range(NCHUNK):
                    filler = None
                    if h == 0 and c == 0:
                        filler = lambda mt: xmn_block(HPG - 1, mt)
                    cur = s_phase(h, c, qT8, pend, filler)
                    pend = cur
            # flush: last chunk's den + AV (merge DMAs the final y^T chunk)
            for i in range(NT):
                av_block(pend, i)
                if i == 0:
                    den_bcast(pend)

    nc.compile()
    return nc


def _ensure_nc():
    if "nc" not in _state:
        _state["nc"] = _build()
    return _state["nc"]


def _make_in_maps(x, Wq, Wp):
    bf = np.float16
    f8 = ml_dtypes.float8_e4m3
    # fold the output projection into per-head value matrices:
    # M_h = Wq_h @ Wp_h  (weight-only, input-independent)
    wms = []
    for hg in range(HG):
        Mi = np.empty((D, JW), np.float32)
        for hh in range(HPG):
            g = hg * HPG + hh
            Mi[:, hh * D : (hh + 1) * D] = (
                Wq[:, g * D : (g + 1) * D] @ Wp[g * D : (g + 1) * D, :]
            )
        wms.append(Mi.astype(bf))
    in_maps = []
    for c in range(NCORES):
        b, hg = c // HG, c % HG
        xt = np.ascontiguousarray(x[b].T)
        in_maps.append({
            "xt": xt.astype(bf),
            "xt8": xt.astype(f8),
            "wq": np.ascontiguousarray(Wq[:, hg * JW : (hg + 1) * JW]).astype(bf),
            "wm": wms[hg],
        })
    return in_maps


def _get_runner():
    """Build once and cache a jitted 8-core runner (avoids re-jit per call)."""
    if "run" in _state:
        return _state["run"]

    import jax
    import concourse.mybir as mybir
    from jax.sharding import Mesh, PartitionSpec
    from jax.experimental.shard_map import shard_map
    from concourse import bass2jax

    nc = _ensure_nc()
    bass2jax.install_neuronx_cc_hook()

    partition_name = nc.partition_id_tensor.name if nc.partition_id_tensor else None
    in_names, out_names, out_avals, zero_outs = [], [], [], []
    for alloc in nc.m.functions[0].allocations:
        if not isinstance(alloc, mybir.MemoryLocationSet):
            continue
        name = alloc.memorylocations[0].name
        if alloc.kind == "ExternalInput":
            if name != partition_name:
                in_names.append(name)
        elif alloc.kind == "ExternalOutput":
            shape = tuple(alloc.tensor_shape)
            dtype = mybir.dt.np(alloc.dtype)
            out_avals.append(jax.core.ShapedArray(shape, dtype))
            out_names.append(name)
            zero_outs.append(np.zeros(shape, dtype))
    n_params = len(in_names)
    n_outs = len(out_names)
    all_in_names = list(in_names) + list(out_names)
    if partition_name is not None:
        all_in_names.append(partition_name)

    def _body(*args):
        operands = list(args)
        if partition_name is not None:
            operands.append(bass2jax.partition_id_tensor())
        outs = bass2jax._bass_exec_p.bind(
            *operands,
            out_avals=tuple(out_avals),
            in_names=tuple(all_in_names),
            out_names=tuple(out_names),
            lowering_input_output_aliases=(),
            sim_require_finite=True,
            sim_require_nnan=True,
            nc=nc,
        )
        return tuple(outs)

    devices = jax.devices()[:NCORES]
    mesh = Mesh(np.asarray(devices), ("core",))
    in_specs = (PartitionSpec("core"),) * (n_params + n_outs)
    out_specs = (PartitionSpec("core"),) * n_outs
    sharded = jax.jit(
        shard_map(_body, mesh=mesh, in_specs=in_specs, out_specs=out_specs,
                  check_rep=False),
        donate_argnums=tuple(range(n_params, n_params + n_outs)),
        keep_unused=True,
    )

    def run(in_maps):
        concat_in = [
            np.concatenate([np.asarray(m[name]) for m in in_maps], axis=0)
            for name in in_names
        ]
        concat_zeros = [
            np.zeros((NCORES * z.shape[0], *z.shape[1:]), z.dtype) for z in zero_outs
        ]
        out_arrs = sharded(*concat_in, *concat_zeros)
        return [
            {
                name: np.asarray(out_arrs[i]).reshape(NCORES, *out_avals[i].shape)[c]
                for i, name in enumerate(out_names)
            }
            for c in range(NCORES)
        ]

    _state["run"] = run
    return run


def kernel(x, Wq, Wv, Wp, bp):
    x = np.asarray(x, np.float32)
    Wq = np.asarray(Wq, np.float32)
    Wp = np.asarray(Wp, np.float32)
    bp = np.asarray(bp, np.float32)

    run = _get_runner()
    results = run(_make_in_maps(x, Wq, Wp))
    y = np.empty((B, N, D), np.float32)
    for b in range(B):
        yt = results[b * HG]["yt"] + results[b * HG + 1]["yt"]
        y[b] = yt.T + bp[None, :]
    return y


# revision 19
# speedup vs baseline: 1.1151x; 1.0013x over previous
"""Multi-head attention V2 kernel for Trainium2 (8 NeuronCores).

Problem shapes (hardcoded): x [4, 2048, 512] f32, Wq [512, 4096], Wv unused,
Wp [4096, 512], bp [512].  Reference math (note: V uses the Q projection):
    q = v = (x @ Wq) -> [B, H, N, D] with H=8, head dim = D = 512
    S = q @ x^T / sqrt(D);  P = softmax(S, -1);  out = (P @ v) @ Wp + bp

Sharding: core = (batch b, head-group hg) with 2 groups of 4 heads.

Weight folding (host): M_h = Wq_h @ Wp_h [D, D] per head, so
    y = sum_h P_h @ (x @ M_h) + bp
which eliminates the device-side output projection: per-head AV matmuls
use xM_h = x @ M_h as the value operand and their PSUM results merge
(scaled by the softmax reciprocal) into a single y^T accumulator in SBUF
on the DVE.  The xM_h tiles are computed on the PE in a prologue that
replaces the old dummy-matmul HAM warmup with real work.

q^T (fp8, scores rhs) is computed directly on the PE as Wq_h^T x^T
(lhsT = wq) and cast f32->fp8 by the DVE -- no token-major q, no DMA
xbar transposes, no SP-queue pressure, no head-boundary cast stalls.

The scores matmul S^T = x q^T runs in fp8e4 DoubleRow mode (K=256 per
instruction); both operands are TRN fp8_e4m3 (xT8 host-converted, qT8
cast on-device from f32 PSUM).  Everything else is fp16 with fp32 PSUM
accumulation; the y^T accumulator and output are f32.

Pipelining: identical slot discipline to V1 -- each s_phase interleaves
chunk c's 16 [DR score pair + exp] slots with the pending chunk's 64 AV
matmuls (1 pair : 4 AV MMs per slot, measured slot 1310ns), denominator
matmuls (ones^T expS, 4-way column-group packs) run at the top of the
following phase, lagging their exps.  qt_phases (64 MMs, exp-free) run
bare between heads; s_phase(0,0), which has no pending AV, interleaves
the last head's xM tiles as filler instead of dummy matmuls.  Head 3's
AV merge DMAs each completed y^T chunk straight out.
"""

import sys

sys.path.insert(0, "/opt/trn_rl_repo")

import numpy as np
import ml_dtypes

B, N, D, H = 4, 2048, 512, 8
NCORES = 8
HG = 2            # head groups (cores per batch)
HPG = H // HG     # heads per core
JW = HPG * D      # per-core Wq column count (2048)
KT = D // 128     # k-tiles over feature dim (4)
NT = N // 128     # partition tiles over tokens (16)
NCHUNK = 4        # n split into 4 chunks of 512
CW = N // NCHUNK  # chunk width (512)
INV_SQRT_D = 1.0 / float(np.sqrt(D))

_state = {}


def _build():
    import concourse.bass as bass
    import concourse.mybir as mybir
    import concourse.tile as tile
    from concourse import bacc

    f32 = mybir.dt.float32
    bf16 = mybir.dt.float16
    f8 = mybir.dt.float8e4
    DR = mybir.MatmulPerfMode.DoubleRow

    nc = bacc.Bacc("TRN2", target_bir_lowering=False)

    xT_d = nc.dram_tensor("xt", [D, N], bf16, kind="ExternalInput")
    xT8_d = nc.dram_tensor("xt8", [D, N], f8, kind="ExternalInput")
    wq_d = nc.dram_tensor("wq", [D, JW], bf16, kind="ExternalInput")
    wm_d = nc.dram_tensor("wm", [D, JW], bf16, kind="ExternalInput")
    yT_d = nc.dram_tensor("yt", [D, N], bf16, kind="ExternalOutput")

    with tile.TileContext(nc) as tc:
        with (
            tc.tile_pool(name="const", bufs=1) as cpool,
            tc.tile_pool(name="qt8", bufs=1) as qt8_pool,
            tc.tile_pool(name="exps", bufs=2) as exps_pool,
            tc.tile_pool(name="rcp", bufs=3) as rcp_pool,
            tc.tile_pool(name="mrg", bufs=5) as mrg_pool,
            tc.tile_pool(name="ps_stage", bufs=2, space="PSUM") as ps_stage,
            tc.tile_pool(name="ps_scores", bufs=3, space="PSUM") as ps_scores,
            tc.tile_pool(name="ps_av", bufs=2, space="PSUM") as ps_av,
            tc.tile_pool(name="ps_den", bufs=1, space="PSUM") as ps_den,
        ):
            # ---- resident inputs ----
            xT = cpool.tile([128, KT, N], bf16, name="xT")
            xT8 = cpool.tile([128, KT, N], f8, name="xT8")
            wq = cpool.tile([128, KT, JW], bf16, name="wq")
            wm = cpool.tile([128, KT, JW], bf16, name="wm")
            # Each hwdge queue sustains only ~82GB/s, so the critical wave
            # is spread by need-time (the prologue interleaves heads 0/1
            # m-tile-wise, halving the xT consumption rate to ~76GB/s):
            #   sync:   xT cn-major (cn needed every ~6.9us)
            #   scalar: wm head 0 (first MM), then wq h-major fine slabs
            #           (head h needed at qt_phase(h))
            #   gpsimd: wm head 1 (~8us), wm head 2 (~35us), xT8 (~60us),
            #           wm head 3 (~65us)
            for cn in range(NCHUNK):
                for k in range(KT):
                    nc.sync.dma_start(
                        xT[:, k, cn * CW : (cn + 1) * CW],
                        xT_d[k * 128 : (k + 1) * 128, cn * CW : (cn + 1) * CW],
                    )
            for k in range(KT):
                nc.scalar.dma_start(
                    wm[:, k, 0:D], wm_d[k * 128 : (k + 1) * 128, 0:D]
                )
            for hh in range(HPG):
                for k in range(KT):
                    nc.scalar.dma_start(
                        wq[:, k, hh * D : (hh + 1) * D],
                        wq_d[k * 128 : (k + 1) * 128, hh * D : (hh + 1) * D],
                    )
            for h in (1, 2):
                for k in range(KT):
                    nc.gpsimd.dma_start(
                        wm[:, k, h * D : (h + 1) * D],
                        wm_d[k * 128 : (k + 1) * 128, h * D : (h + 1) * D],
                    )
            for k in range(KT):
                nc.gpsimd.dma_start(xT8[:, k, :], xT8_d[k * 128 : (k + 1) * 128, :])
            for k in range(KT):
                nc.gpsimd.dma_start(
                    wm[:, k, (HPG - 1) * D : HPG * D],
                    wm_d[k * 128 : (k + 1) * 128, (HPG - 1) * D : HPG * D],
                )

            ones_col = cpool.tile([128, 1], bf16, name="ones_col")
            nc.vector.memset(ones_col[:, :], 1.0)
            # touch Exp once during the input-DMA wait so the ~2.7us ACT
            # table-set load is off the first chunk's critical path
            nc.scalar.activation(
                ones_col[:, :], ones_col[:, :],
                mybir.ActivationFunctionType.Exp, scale=0.0,
            )
            # f32r inputs to the sum+broadcast matmul must be produced by
            # "rounding" writes, so stage through a scratch tile
            f32r = mybir.dt.float32r
            ones128 = cpool.tile([128, 128], f32r, name="ones128")
            scr = mrg_pool.tile([128, CW], bf16, name="scr", tag="mrg")
            nc.vector.memset(scr[:, :], 1.0)
            nc.vector.tensor_copy(ones128[:, :], scr[:, 0:128])

            # per-head xM tiles (token-major values) and per-chunk y^T
            # accumulators
            xmn = [
                cpool.tile([128, NT, D], bf16, name=f"xmn{h}") for h in range(HPG)
            ]
            ysb = [
                cpool.tile([128, KT, CW], bf16, name=f"ysb{c}")
                for c in range(NCHUNK)
            ]

            # ---------- pipelined emission helpers ----------
            def xmn_block(h, mt):
                """xM_h token-major tile mt: 4 accumulation MMs + DVE cast."""
                ps = ps_stage.tile([128, D], f32, name="ps_x", tag="stage")
                for k in range(KT):
                    nc.tensor.matmul(
                        ps[:, :],
                        lhsT=xT[:, k, mt * 128 : (mt + 1) * 128],
                        rhs=wm[:, k, h * D : (h + 1) * D],
                        start=(k == 0),
                        stop=(k == KT - 1),
                    )
                nc.vector.tensor_copy(xmn[h][:, mt, :], ps[:, :])

            def av_block(p, i):
                """Slot i of 16: 4 AV accumulation MMs for pending chunk p
                (dt = i//4, m-tiles 4*(i%4)..+4); when a dt completes, merge
                the scaled result into the y^T accumulator (and DMA it out
                on the last head)."""
                if p is None:
                    return
                dt, m0 = i // 4, 4 * (i % 4)
                if m0 == 0:
                    p["ps"] = ps_av.tile([128, CW], f32, name="ps_av", tag="av")
                for mt in range(m0, m0 + 4):
                    nc.tensor.matmul(
                        p["ps"][:, :],
                        lhsT=p["xmn"][:, mt, dt * 128 : (dt + 1) * 128],
                        rhs=p["expS"][:, mt, :],
                        start=(mt == 0),
                        stop=(mt == NT - 1),
                    )
                if m0 + 4 == NT:
                    dst = ysb[p["c"]][:, dt, :]
                    if p["h"] == 0:
                        nc.vector.tensor_mul(dst, p["ps"][:, :], p["rcpB"][:, :])
                    else:
                        t = mrg_pool.tile([128, CW], bf16, name="mrg", tag="mrg")
                        nc.vector.tensor_mul(t[:, :], p["ps"][:, :], p["rcpB"][:, :])
                        nc.vector.tensor_add(dst, dst, t[:, :])
                    if p["h"] == HPG - 1:
                        nc.sync.dma_start(
                            yT_d[dt * 128 : (dt + 1) * 128,
                                 p["n0"] : p["n0"] + CW],
                            dst,
                        )

            def den_bcast(p):
                """Denominator finish for the pending chunk, emitted one
                slot into the NEXT phase: the 16 expS m-tiles were summed
                elementwise by a DVE running chain during p's own phase
                (p["zp"], f32r); one K=128 sum+broadcast matmul and the
                reciprocal produce rcpB."""
                if p is None or "zp" not in p:
                    return
                psb = ps_den.tile([128, CW], f32, name="psb", tag="den")
                nc.tensor.matmul(
                    psb[:, :], lhsT=ones128[:, :], rhs=p["zp"][:, :],
                    start=True, stop=True,
                )
                nc.vector.reciprocal_approx_fast(p["rcpB"][:, :], psb[:, :])
                del p["zp"]

            def qt_phase(h, pend):
                """q^T (fp8) for head h, computed directly on the PE as
                Wq_h^T x^T: 16 stages of 4 accumulation MMs + f32->fp8 DVE
                cast.  Runs bare (exp-independent, dense); the pending AV
                passes through to the following s_phase(h,0).  cn-major
                order so chunk 0's scores operands land first."""
                qT8 = qt8_pool.tile([128, KT, N], f8, name="qT8", tag="qT8")
                first = True
                for cn in range(NCHUNK):
                    for jb in range(KT):
                        ps = ps_stage.tile([128, CW], f32, name="ps_q", tag="stage")
                        for k in range(KT):
                            nc.tensor.matmul(
                                ps[:, :],
                                lhsT=wq[:, k, h * D + jb * 128 : h * D + (jb + 1) * 128],
                                rhs=xT[:, k, cn * CW : (cn + 1) * CW],
                                start=(k == 0),
                                stop=(k == KT - 1),
                            )
                        nc.vector.tensor_copy(qT8[:, jb, cn * CW : (cn + 1) * CW], ps[:, :])
                        if first:
                            den_bcast(pend)
                            first = False
                return qT8

            def s_phase(h, c, qT8, pend, filler=None):
                """Scores phase for chunk (h, c): 16 slots of [DR score
                pair + exp] interleaved with pend's AV; den groups for this
                chunk lag their exps by >=2 slots.  filler (s_phase(0,0)
                only, where no pending AV exists) emits real PE work to
                keep the slot structure dense."""
                n0 = c * CW
                expS = exps_pool.tile([128, NT, CW], bf16, name="expS", tag="expS")
                cur = {
                    "expS": expS, "xmn": xmn[h], "h": h, "c": c, "n0": n0,
                    "rcpB": rcp_pool.tile([128, CW], f32, name="rcpB", tag="rcpB"),
                }
                for mt in range(NT):
                    ps = ps_scores.tile([128, CW], f32, name="ps_s", tag="scores")
                    for t in range(KT // 2):
                        nc.tensor.matmul(
                            ps[:, :],
                            lhsT=xT8[:, 2 * t : 2 * t + 2, mt * 128 : (mt + 1) * 128],
                            rhs=qT8[:, 2 * t : 2 * t + 2, n0 : n0 + CW],
                            start=(t == 0),
                            stop=(t == KT // 2 - 1),
                            perf_mode=DR,
                        )
                    nc.scalar.activation(
                        expS[:, mt, :], ps[:, :],
                        mybir.ActivationFunctionType.Exp, scale=INV_SQRT_D,
                    )
                    # running elementwise sum of the expS m-tiles (DVE):
                    # replaces the packed PE column-sum matmuls
                    if mt == 1:
                        acc = mrg_pool.tile([128, CW], bf16, name="acc", tag="mrg")
                        cur["acc"] = acc
                        nc.vector.tensor_add(
                            acc[:, :], expS[:, 0, :], expS[:, 1, :]
                        )
                    elif mt >= 2:
                        acc = cur["acc"]
                        nc.vector.tensor_add(acc[:, :], acc[:, :], expS[:, mt, :])
                    av_block(pend, mt)
                    if filler is not None:
                        filler(mt)
                    if mt == 0:
                        den_bcast(pend)
                zp = rcp_pool.tile([128, CW], mybir.dt.float32r, name="zp", tag="zp")
                nc.vector.tensor_copy(zp[:, :], cur["acc"][:, :])
                cur["zp"] = zp
                return cur

            # ---------- the pipeline ----------
            # prologue: xM for heads 0..2 doubles as the HAM warmup (real
            # work from the first instruction); head 3's xM is s_phase(0,0)
            # filler.  Heads 0/1 interleave m-tile-wise so each xT column
            # chunk is consumed at half rate (one DMA queue keeps up).
            for mt in range(NT):
                xmn_block(0, mt)
                xmn_block(1, mt)
            for mt in range(NT):
                xmn_block(2, mt)
            pend = None
            for h in range(HPG):
                qT8 = qt_phase(h, pend)
                for c in range(NCHUNK):
                    filler = None
                    if h == 0 and c == 0:
                        filler = lambda mt: xmn_block(HPG - 1, mt)
                    cur = s_phase(h, c, qT8, pend, filler)
                    pend = cur
            # flush: last chunk's den + AV (merge DMAs the final y^T chunk)
            for i in range(NT):
                av_block(pend, i)
                if i == 0:
                    den_bcast(pend)

    nc.compile()
    return nc


def _ensure_nc():
    if "nc" not in _state:
        _state["nc"] = _build()
    return _state["nc"]


def _make_in_maps(x, Wq, Wp):
    bf = np.float16
    f8 = ml_dtypes.float8_e4m3
    # fold the output projection into per-head value matrices:
    # M_h = Wq_h @ Wp_h  (weight-only, input-independent)
    wms = []
    for hg in range(HG):
        Mi = np.empty((D, JW), np.float32)
        for hh in range(HPG):
            g = hg * HPG + hh
            Mi[:, hh * D : (hh + 1) * D] = (
                Wq[:, g * D : (g + 1) * D] @ Wp[g * D : (g + 1) * D, :]
            )
        wms.append(Mi.astype(bf))
    in_maps = []
    for c in range(NCORES):
        b, hg = c // HG, c % HG
        xt = np.ascontiguousarray(x[b].T)
        in_maps.append({
            "xt": xt.astype(bf),
            "xt8": xt.astype(f8),
            "wq": np.ascontiguousarray(Wq[:, hg * JW : (hg + 1) * JW]).astype(bf),
            "wm": wms[hg],
        })
    return in_maps


def _get_runner():
    """Build once and cache a jitted 8-core runner (avoids re-jit per call)."""
    if "run" in _state:
        return _state["run"]

    import jax
    import concourse.mybir as mybir
    from jax.sharding import Mesh, PartitionSpec
    from jax.experimental.shard_map import shard_map
    from concourse import bass2jax

    nc = _ensure_nc()
    bass2jax.install_neuronx_cc_hook()

    partition_name = nc.partition_id_tensor.name if nc.partition_id_tensor else None
    in_names, out_names, out_avals, zero_outs = [], [], [], []
    for alloc in nc.m.functions[0].allocations:
        if not isinstance(alloc, mybir.MemoryLocationSet):
            continue
        name = alloc.memorylocations[0].name
        if alloc.kind == "ExternalInput":
            if name != partition_name:
                in_names.append(name)
        elif alloc.kind == "ExternalOutput":
            shape = tuple(alloc.tensor_shape)
            dtype = mybir.dt.np(alloc.dtype)
            out_avals.append(jax.core.ShapedArray(shape, dtype))
            out_names.append(name)
            zero_outs.append(np.zeros(shape, dtype))
    n_params = len(in_names)
    n_outs = len(out_names)
    all_in_names = list(in_names) + list(out_names)
    if partition_name is not None:
        all_in_names.append(partition_name)

    def _body(*args):
        operands = list(args)
        if partition_name is not None:
            operands.append(bass2jax.partition_id_tensor())
        outs = bass2jax._bass_exec_p.bind(
            *operands,
            out_avals=tuple(out_avals),
            in_names=tuple(all_in_names),
            out_names=tuple(out_names),
            lowering_input_output_aliases=(),
            sim_require_finite=True,
            sim_require_nnan=True,
            nc=nc,
        )
        return tuple(outs)

    devices = jax.devices()[:NCORES]
    mesh = Mesh(np.asarray(devices), ("core",))
    in_specs = (PartitionSpec("core"),) * (n_params + n_outs)
    out_specs = (PartitionSpec("core"),) * n_outs
    sharded = jax.jit(
        shard_map(_body, mesh=mesh, in_specs=in_specs, out_specs=out_specs,
                  check_rep=False),
        donate_argnums=tuple(range(n_params, n_params + n_outs)),
        keep_unused=True,
    )

    def run(in_maps):
        concat_in = [
            np.concatenate([np.asarray(m[name]) for m in in_maps], axis=0)
            for name in in_names
        ]
        concat_zeros = [
            np.zeros((NCORES * z.shape[0], *z.shape[1:]), z.dtype) for z in zero_outs
        ]
        out_arrs = sharded(*concat_in, *concat_zeros)
        return [
            {
                name: np.asarray(out_arrs[i]).reshape(NCORES, *out_avals[i].shape)[c]
                for i, name in enumerate(out_names)
            }
            for c in range(NCORES)
        ]

    _state["run"] = run
    return run


def kernel(x, Wq, Wv, Wp, bp):
    x = np.asarray(x, np.float32)
    Wq = np.asarray(Wq, np.float32)
    Wp = np.asarray(Wp, np.float32)
    bp = np.asarray(bp, np.float32)

    run = _get_runner()
    results = run(_make_in_maps(x, Wq, Wp))
    y = np.empty((B, N, D), np.float32)
    for b in range(B):
        yt = (results[b * HG]["yt"].astype(np.float32)
              + results[b * HG + 1]["yt"].astype(np.float32))
        y[b] = yt.T + bp[None, :]
    return y


# revision 23
# speedup vs baseline: 1.1239x; 1.0079x over previous
"""Multi-head attention V2 kernel for Trainium2 (8 NeuronCores).

Problem shapes (hardcoded): x [4, 2048, 512] f32, Wq [512, 4096], Wv unused,
Wp [4096, 512], bp [512].  Reference math (note: V uses the Q projection):
    q = v = (x @ Wq) -> [B, H, N, D] with H=8, head dim = D = 512
    S = q @ x^T / sqrt(D);  P = softmax(S, -1);  out = (P @ v) @ Wp + bp

Sharding: core = (batch b, head-group hg) with 2 groups of 4 heads.

Weight folding (host): M_h = Wq_h @ Wp_h [D, D] per head, so
    y = sum_h P_h @ (x @ M_h) + bp
which eliminates the device-side output projection: per-head AV matmuls
use xM_h = x @ M_h as the value operand and their PSUM results merge
(scaled by the softmax reciprocal) into a single y^T accumulator in SBUF
on the DVE.  The xM_h tiles are computed on the PE in a prologue that
replaces the old dummy-matmul HAM warmup with real work.

q^T (fp8, scores rhs) is computed directly on the PE as Wq_h^T x^T
(lhsT = wq) and cast f32->fp8 by the DVE -- no token-major q, no DMA
xbar transposes, no SP-queue pressure, no head-boundary cast stalls.

The scores matmul S^T = x q^T runs in fp8e4 DoubleRow mode (K=256 per
instruction); both operands are TRN fp8_e4m3 (xT8 host-converted, qT8
cast on-device from f32 PSUM).  Everything else is fp16 with fp32 PSUM
accumulation; the y^T accumulator and output are fp16.

Pipelining: each s_phase interleaves chunk c's 16 [DR score pair + exp]
slots with the pending chunk's 64 AV matmuls (1 pair : 4 AV MMs per
slot, measured slot 1310ns).  The softmax denominator is a DVE running
elementwise sum of the expS m-tiles (off the PE); one K=128 f32r
sum+broadcast matmul + reciprocal at the top of the following phase
produce rcpB before the first AV merge needs it.  qt_phases (64 MMs,
exp-free) run bare between heads; s_phase(0,0), which has no pending
AV, interleaves head 3's xM tiles as filler.  Head 3's AV merges DMA
each completed y^T chunk straight out; the final chunk merges in halves
to shorten the end-of-kernel serial chain.  Input DMA is spread across
the sync/scalar/gpsimd hwdge queues by need-time (~82GB/s per queue).
"""

import sys

sys.path.insert(0, "/opt/trn_rl_repo")

import numpy as np
import ml_dtypes

B, N, D, H = 4, 2048, 512, 8
NCORES = 8
HG = 2            # head groups (cores per batch)
HPG = H // HG     # heads per core
JW = HPG * D      # per-core Wq column count (2048)
KT = D // 128     # k-tiles over feature dim (4)
NT = N // 128     # partition tiles over tokens (16)
NCHUNK = 4        # n split into 4 chunks of 512
CW = N // NCHUNK  # chunk width (512)
INV_SQRT_D = 1.0 / float(np.sqrt(D))

_state = {}


def _build():
    import concourse.bass as bass
    import concourse.mybir as mybir
    import concourse.tile as tile
    from concourse import bacc

    f32 = mybir.dt.float32
    bf16 = mybir.dt.float16
    f8 = mybir.dt.float8e4
    DR = mybir.MatmulPerfMode.DoubleRow

    nc = bacc.Bacc("TRN2", target_bir_lowering=False)

    xT_d = nc.dram_tensor("xt", [D, N], bf16, kind="ExternalInput")
    xT8_d = nc.dram_tensor("xt8", [D, N], f8, kind="ExternalInput")
    wq_d = nc.dram_tensor("wq", [D, JW], bf16, kind="ExternalInput")
    wm_d = nc.dram_tensor("wm", [D, JW], bf16, kind="ExternalInput")
    yT_d = nc.dram_tensor("yt", [D, N], bf16, kind="ExternalOutput")

    with tile.TileContext(nc) as tc:
        with (
            tc.tile_pool(name="const", bufs=1) as cpool,
            tc.tile_pool(name="qt8", bufs=1) as qt8_pool,
            tc.tile_pool(name="exps", bufs=2) as exps_pool,
            tc.tile_pool(name="rcp", bufs=3) as rcp_pool,
            tc.tile_pool(name="mrg", bufs=5) as mrg_pool,
            tc.tile_pool(name="ps_stage", bufs=2, space="PSUM") as ps_stage,
            tc.tile_pool(name="ps_scores", bufs=3, space="PSUM") as ps_scores,
            tc.tile_pool(name="ps_av", bufs=2, space="PSUM") as ps_av,
            tc.tile_pool(name="ps_den", bufs=1, space="PSUM") as ps_den,
        ):
            # ---- resident inputs ----
            xT = cpool.tile([128, KT, N], bf16, name="xT")
            xT8 = cpool.tile([128, KT, N], f8, name="xT8")
            wq = cpool.tile([128, KT, JW], bf16, name="wq")
            wm = cpool.tile([128, KT, JW], bf16, name="wm")
            # Each hwdge queue sustains only ~82GB/s, so the critical wave
            # is spread by need-time (the prologue interleaves heads 0/1
            # m-tile-wise, halving the xT consumption rate to ~76GB/s):
            #   sync:   xT cn-major (cn needed every ~6.9us)
            #   scalar: wm head 0 (first MM), then wq h-major fine slabs
            #           (head h needed at qt_phase(h))
            #   gpsimd: wm head 1 (~8us), wm head 2 (~35us), xT8 (~60us),
            #           wm head 3 (~65us)
            for cn in range(NCHUNK):
                for k in range(KT):
                    nc.sync.dma_start(
                        xT[:, k, cn * CW : (cn + 1) * CW],
                        xT_d[k * 128 : (k + 1) * 128, cn * CW : (cn + 1) * CW],
                    )
            for k in range(KT):
                nc.scalar.dma_start(
                    wm[:, k, 0:D], wm_d[k * 128 : (k + 1) * 128, 0:D]
                )
            for hh in range(HPG):
                for k in range(KT):
                    nc.scalar.dma_start(
                        wq[:, k, hh * D : (hh + 1) * D],
                        wq_d[k * 128 : (k + 1) * 128, hh * D : (hh + 1) * D],
                    )
            for h in (1, 2):
                for k in range(KT):
                    nc.gpsimd.dma_start(
                        wm[:, k, h * D : (h + 1) * D],
                        wm_d[k * 128 : (k + 1) * 128, h * D : (h + 1) * D],
                    )
            for k in range(KT):
                nc.gpsimd.dma_start(xT8[:, k, :], xT8_d[k * 128 : (k + 1) * 128, :])
            for k in range(KT):
                nc.gpsimd.dma_start(
                    wm[:, k, (HPG - 1) * D : HPG * D],
                    wm_d[k * 128 : (k + 1) * 128, (HPG - 1) * D : HPG * D],
                )

            # small PE warmup: ~8 dummy matmuls (~3.4us cold) give the DMA
            # queues a head start so the xM prologue never outruns delivery
            warm = cpool.tile([128, CW], bf16, name="warm")
            nc.vector.memset(warm[:, :], 1.0)
            ps_w = ps_stage.tile([128, CW], f32, name="ps_w", tag="stage")
            for _ in range(8):
                nc.tensor.matmul(
                    ps_w[:, :], lhsT=warm[:, 0:128], rhs=warm[:, :],
                    start=True, stop=True,
                )

            ones_col = cpool.tile([128, 1], bf16, name="ones_col")
            nc.vector.memset(ones_col[:, :], 1.0)
            # touch Exp once during the input-DMA wait so the ~2.7us ACT
            # table-set load is off the first chunk's critical path
            nc.scalar.activation(
                ones_col[:, :], ones_col[:, :],
                mybir.ActivationFunctionType.Exp, scale=0.0,
            )
            # f32r inputs to the sum+broadcast matmul must be produced by
            # "rounding" writes, so stage through a scratch tile
            f32r = mybir.dt.float32r
            ones128 = cpool.tile([128, 128], f32r, name="ones128")
            scr = mrg_pool.tile([128, CW], bf16, name="scr", tag="mrg")
            nc.vector.memset(scr[:, :], 1.0)
            nc.vector.tensor_copy(ones128[:, :], scr[:, 0:128])

            # per-head xM tiles (token-major values) and per-chunk y^T
            # accumulators
            xmn = [
                cpool.tile([128, NT, D], bf16, name=f"xmn{h}") for h in range(HPG)
            ]
            ysb = [
                cpool.tile([128, KT, CW], bf16, name=f"ysb{c}")
                for c in range(NCHUNK)
            ]

            # ---------- pipelined emission helpers ----------
            def xmn_block(h, mt):
                """xM_h token-major tile mt: 4 accumulation MMs + DVE cast."""
                ps = ps_stage.tile([128, D], f32, name="ps_x", tag="stage")
                for k in range(KT):
                    nc.tensor.matmul(
                        ps[:, :],
                        lhsT=xT[:, k, mt * 128 : (mt + 1) * 128],
                        rhs=wm[:, k, h * D : (h + 1) * D],
                        start=(k == 0),
                        stop=(k == KT - 1),
                    )
                nc.vector.tensor_copy(xmn[h][:, mt, :], ps[:, :])

            def av_block(p, i):
                """Slot i of 16: 4 AV accumulation MMs for pending chunk p
                (dt = i//4, m-tiles 4*(i%4)..+4); when a dt completes, merge
                the scaled result into the y^T accumulator (and DMA it out
                on the last head)."""
                if p is None:
                    return
                dt, m0 = i // 4, 4 * (i % 4)
                if m0 == 0:
                    p["ps"] = ps_av.tile([128, CW], f32, name="ps_av", tag="av")
                for mt in range(m0, m0 + 4):
                    nc.tensor.matmul(
                        p["ps"][:, :],
                        lhsT=p["xmn"][:, mt, dt * 128 : (dt + 1) * 128],
                        rhs=p["expS"][:, mt, :],
                        start=(mt == 0),
                        stop=(mt == NT - 1),
                    )
                if m0 + 4 == NT:
                    dst = ysb[p["c"]][:, dt, :]
                    if p["h"] == 0:
                        nc.vector.tensor_mul(dst, p["ps"][:, :], p["rcpB"][:, :])
                    elif p["h"] == HPG - 1 and p["c"] == NCHUNK - 1:
                        # final chunk: merge + DMA in halves so the
                        # end-of-kernel serial chain is ~0.7us shorter
                        t = mrg_pool.tile([128, CW], bf16, name="mrg", tag="mrg")
                        hw = CW // 2
                        for half in range(2):
                            lo, hi = half * hw, (half + 1) * hw
                            dsth = ysb[p["c"]][:, dt, lo:hi]
                            nc.vector.tensor_mul(
                                t[:, lo:hi], p["ps"][:, lo:hi], p["rcpB"][:, lo:hi]
                            )
                            nc.vector.tensor_add(dsth, dsth, t[:, lo:hi])
                            nc.sync.dma_start(
                                yT_d[dt * 128 : (dt + 1) * 128,
                                     p["n0"] + lo : p["n0"] + hi],
                                dsth,
                            )
                        return
                    else:
                        t = mrg_pool.tile([128, CW], bf16, name="mrg", tag="mrg")
                        nc.vector.tensor_mul(t[:, :], p["ps"][:, :], p["rcpB"][:, :])
                        nc.vector.tensor_add(dst, dst, t[:, :])
                    if p["h"] == HPG - 1:
                        nc.sync.dma_start(
                            yT_d[dt * 128 : (dt + 1) * 128,
                                 p["n0"] : p["n0"] + CW],
                            dst,
                        )

            def den_bcast(p):
                """Denominator finish for the pending chunk, emitted one
                slot into the NEXT phase: the 16 expS m-tiles were summed
                elementwise by a DVE running chain during p's own phase
                (p["zp"], f32r); one K=128 sum+broadcast matmul and the
                reciprocal produce rcpB."""
                if p is None or "zp" not in p:
                    return
                psb = ps_den.tile([128, CW], f32, name="psb", tag="den")
                nc.tensor.matmul(
                    psb[:, :], lhsT=ones128[:, :], rhs=p["zp"][:, :],
                    start=True, stop=True,
                )
                nc.vector.reciprocal_approx_fast(p["rcpB"][:, :], psb[:, :])
                del p["zp"]

            def qt_phase(h, pend):
                """q^T (fp8) for head h, computed directly on the PE as
                Wq_h^T x^T: 16 stages of 4 accumulation MMs + f32->fp8 DVE
                cast.  Runs bare (exp-independent, dense); the pending AV
                passes through to the following s_phase(h,0).  cn-major
                order so chunk 0's scores operands land first."""
                qT8 = qt8_pool.tile([128, KT, N], f8, name="qT8", tag="qT8")
                first = True
                for cn in range(NCHUNK):
                    for jb in range(KT):
                        ps = ps_stage.tile([128, CW], f32, name="ps_q", tag="stage")
                        for k in range(KT):
                            nc.tensor.matmul(
                                ps[:, :],
                                lhsT=wq[:, k, h * D + jb * 128 : h * D + (jb + 1) * 128],
                                rhs=xT[:, k, cn * CW : (cn + 1) * CW],
                                start=(k == 0),
                                stop=(k == KT - 1),
                            )
                        nc.vector.tensor_copy(qT8[:, jb, cn * CW : (cn + 1) * CW], ps[:, :])
                        if first:
                            den_bcast(pend)
                            first = False
                return qT8

            def s_phase(h, c, qT8, pend, filler=None):
                """Scores phase for chunk (h, c): 16 slots of [DR score
                pair + exp] interleaved with pend's AV; den groups for this
                chunk lag their exps by >=2 slots.  filler (s_phase(0,0)
                only, where no pending AV exists) emits real PE work to
                keep the slot structure dense."""
                n0 = c * CW
                expS = exps_pool.tile([128, NT, CW], bf16, name="expS", tag="expS")
                cur = {
                    "expS": expS, "xmn": xmn[h], "h": h, "c": c, "n0": n0,
                    "rcpB": rcp_pool.tile([128, CW], f32, name="rcpB", tag="rcpB"),
                }
                for mt in range(NT):
                    ps = ps_scores.tile([128, CW], f32, name="ps_s", tag="scores")
                    for t in range(KT // 2):
                        nc.tensor.matmul(
                            ps[:, :],
                            lhsT=xT8[:, 2 * t : 2 * t + 2, mt * 128 : (mt + 1) * 128],
                            rhs=qT8[:, 2 * t : 2 * t + 2, n0 : n0 + CW],
                            start=(t == 0),
                            stop=(t == KT // 2 - 1),
                            perf_mode=DR,
                        )
                    nc.scalar.activation(
                        expS[:, mt, :], ps[:, :],
                        mybir.ActivationFunctionType.Exp, scale=INV_SQRT_D,
                    )
                    # running elementwise sum of the expS m-tiles (DVE):
                    # replaces the packed PE column-sum matmuls
                    if mt == 1:
                        acc = mrg_pool.tile([128, CW], bf16, name="acc", tag="mrg")
                        cur["acc"] = acc
                        nc.vector.tensor_add(
                            acc[:, :], expS[:, 0, :], expS[:, 1, :]
                        )
                    elif mt >= 2:
                        acc = cur["acc"]
                        nc.vector.tensor_add(acc[:, :], acc[:, :], expS[:, mt, :])
                    av_block(pend, mt)
                    if filler is not None:
                        filler(mt)
                    if mt == 0:
                        den_bcast(pend)
                zp = rcp_pool.tile([128, CW], mybir.dt.float32r, name="zp", tag="zp")
                nc.vector.tensor_copy(zp[:, :], cur["acc"][:, :])
                cur["zp"] = zp
                return cur

            # ---------- the pipeline ----------
            # prologue: xM for heads 0..2 doubles as the HAM warmup (real
            # work from the first instruction); head 3's xM is s_phase(0,0)
            # filler.  Heads 0/1 interleave m-tile-wise so each xT column
            # chunk is consumed at half rate (one DMA queue keeps up).
            for mt in range(NT):
                xmn_block(0, mt)
                xmn_block(1, mt)
            for mt in range(NT):
                xmn_block(2, mt)
            pend = None
            for h in range(HPG):
                qT8 = qt_phase(h, pend)
                for c in range(NCHUNK):
                    filler = None
                    if h == 0 and c == 0:
                        filler = lambda mt: xmn_block(HPG - 1, mt)
                    cur = s_phase(h, c, qT8, pend, filler)
                    pend = cur
            # flush: last chunk's den + AV (merge DMAs the final y^T chunk)
            for i in range(NT):
                av_block(pend, i)
                if i == 0:
                    den_bcast(pend)

    nc.compile()
    return nc


def _ensure_nc():
    if "nc" not in _state:
        _state["nc"] = _build()
    return _state["nc"]


def _make_in_maps(x, Wq, Wp):
    bf = np.float16
    f8 = ml_dtypes.float8_e4m3
    # fold the output projection into per-head value matrices:
    # M_h = Wq_h @ Wp_h  (weight-only, input-independent)
    wms = []
    for hg in range(HG):
        Mi = np.empty((D, JW), np.float32)
        for hh in range(HPG):
            g = hg * HPG + hh
            Mi[:, hh * D : (hh + 1) * D] = (
                Wq[:, g * D : (g + 1) * D] @ Wp[g * D : (g + 1) * D, :]
            )
        wms.append(Mi.astype(bf))
    in_maps = []
    for c in range(NCORES):
        b, hg = c // HG, c % HG
        xt = np.ascontiguousarray(x[b].T)
        in_maps.append({
            "xt": xt.astype(bf),
            "xt8": xt.astype(f8),
            "wq": np.ascontiguousarray(Wq[:, hg * JW : (hg + 1) * JW]).astype(bf),
            "wm": wms[hg],
        })
    return in_maps


def _get_runner():
    """Build once and cache a jitted 8-core runner (avoids re-jit per call)."""
    if "run" in _state:
        return _state["run"]

    import jax
    import concourse.mybir as mybir
    from jax.sharding import Mesh, PartitionSpec
    from jax.experimental.shard_map import shard_map
    from concourse import bass2jax

    nc = _ensure_nc()
    bass2jax.install_neuronx_cc_hook()

    partition_name = nc.partition_id_tensor.name if nc.partition_id_tensor else None
    in_names, out_names, out_avals, zero_outs = [], [], [], []
    for alloc in nc.m.functions[0].allocations:
        if not isinstance(alloc, mybir.MemoryLocationSet):
            continue
        name = alloc.memorylocations[0].name
        if alloc.kind == "ExternalInput":
            if name != partition_name:
                in_names.append(name)
        elif alloc.kind == "ExternalOutput":
            shape = tuple(alloc.tensor_shape)
            dtype = mybir.dt.np(alloc.dtype)
            out_avals.append(jax.core.ShapedArray(shape, dtype))
            out_names.append(name)
            zero_outs.append(np.zeros(shape, dtype))
    n_params = len(in_names)
    n_outs = len(out_names)
    all_in_names = list(in_names) + list(out_names)
    if partition_name is not None:
        all_in_names.append(partition_name)

    def _body(*args):
        operands = list(args)
        if partition_name is not None:
            operands.append(bass2jax.partition_id_tensor())
        outs = bass2jax._bass_exec_p.bind(
            *operands,
            out_avals=tuple(out_avals),
            in_names=tuple(all_in_names),
            out_names=tuple(out_names),
            lowering_input_output_aliases=(),
            sim_require_finite=True,
            sim_require_nnan=True,
            nc=nc,
        )
        return tuple(outs)

    devices = jax.devices()[:NCORES]
    mesh = Mesh(np.asarray(devices), ("core",))
    in_specs = (PartitionSpec("core"),) * (n_params + n_outs)
    out_specs = (PartitionSpec("core"),) * n_outs
    sharded = jax.jit(
        shard_map(_body, mesh=mesh, in_specs=in_specs, out_specs=out_specs,
                  check_rep=False),
        donate_argnums=tuple(range(n_params, n_params + n_outs)),
        keep_unused=True,
    )

    def run(in_maps):
        concat_in = [
            np.concatenate([np.asarray(m[name]) for m in in_maps], axis=0)
            for name in in_names
        ]
        concat_zeros = [
            np.zeros((NCORES * z.shape[0], *z.shape[1:]), z.dtype) for z in zero_outs
        ]
        out_arrs = sharded(*concat_in, *concat_zeros)
        return [
            {
                name: np.asarray(out_arrs[i]).reshape(NCORES, *out_avals[i].shape)[c]
                for i, name in enumerate(out_names)
            }
            for c in range(NCORES)
        ]

    _state["run"] = run
    return run


def kernel(x, Wq, Wv, Wp, bp):
    x = np.asarray(x, np.float32)
    Wq = np.asarray(Wq, np.float32)
    Wp = np.asarray(Wp, np.float32)
    bp = np.asarray(bp, np.float32)

    run = _get_runner()
    results = run(_make_in_maps(x, Wq, Wp))
    y = np.empty((B, N, D), np.float32)
    for b in range(B):
        yt = (results[b * HG]["yt"].astype(np.float32)
              + results[b * HG + 1]["yt"].astype(np.float32))
        y[b] = yt.T + bp[None, :]
    return y


# revision 24
# speedup vs baseline: 1.1291x; 1.0046x over previous
"""Multi-head attention V2 kernel for Trainium2 (8 NeuronCores).

Problem shapes (hardcoded): x [4, 2048, 512] f32, Wq [512, 4096], Wv unused,
Wp [4096, 512], bp [512].  Reference math (note: V uses the Q projection):
    q = v = (x @ Wq) -> [B, H, N, D] with H=8, head dim = D = 512
    S = q @ x^T / sqrt(D);  P = softmax(S, -1);  out = (P @ v) @ Wp + bp

Sharding: core = (batch b, head-group hg) with 2 groups of 4 heads.

Weight folding (host): M_h = Wq_h @ Wp_h [D, D] per head, so
    y = sum_h P_h @ (x @ M_h) + bp
which eliminates the device-side output projection: per-head AV matmuls
use xM_h = x @ M_h as the value operand and their PSUM results merge
(scaled by the softmax reciprocal) into a single y^T accumulator in SBUF
on the DVE.  The xM_h tiles are computed on the PE in a prologue that
replaces the old dummy-matmul HAM warmup with real work.

q^T (fp8, scores rhs) is computed directly on the PE as Wq_h^T x^T
(lhsT = wq) and cast f32->fp8 by the DVE -- no token-major q, no DMA
xbar transposes, no SP-queue pressure, no head-boundary cast stalls.

The scores matmul S^T = x q^T runs in fp8e4 DoubleRow mode (K=256 per
instruction); both operands are TRN fp8_e4m3 (xT8 host-converted, qT8
cast on-device from f32 PSUM).  Everything else is fp16 with fp32 PSUM
accumulation; the y^T accumulator and output are fp16.

Pipelining: each s_phase interleaves chunk c's 16 [DR score pair + exp]
slots with the pending chunk's 64 AV matmuls (1 pair : 4 AV MMs per
slot, measured slot 1310ns).  The softmax denominator is a DVE running
elementwise sum of the expS m-tiles (off the PE); one K=128 f32r
sum+broadcast matmul + reciprocal at the top of the following phase
produce rcpB before the first AV merge needs it.  qt_phases (64 MMs,
exp-free) run bare between heads; s_phase(0,0), which has no pending
AV, interleaves head 3's xM tiles as filler.  Head 3's AV merges DMA
each completed y^T chunk straight out; the final chunk merges in halves
to shorten the end-of-kernel serial chain.  Input DMA is spread across
the sync/scalar/gpsimd hwdge queues by need-time (~82GB/s per queue).
"""

import sys

sys.path.insert(0, "/opt/trn_rl_repo")

import numpy as np
import ml_dtypes

B, N, D, H = 4, 2048, 512, 8
NCORES = 8
HG = 2            # head groups (cores per batch)
HPG = H // HG     # heads per core
JW = HPG * D      # per-core Wq column count (2048)
KT = D // 128     # k-tiles over feature dim (4)
NT = N // 128     # partition tiles over tokens (16)
NCHUNK = 4        # n split into 4 chunks of 512
CW = N // NCHUNK  # chunk width (512)
INV_SQRT_D = 1.0 / float(np.sqrt(D))

_state = {}


def _build():
    import concourse.bass as bass
    import concourse.mybir as mybir
    import concourse.tile as tile
    from concourse import bacc

    f32 = mybir.dt.float32
    bf16 = mybir.dt.float16
    f8 = mybir.dt.float8e4
    DR = mybir.MatmulPerfMode.DoubleRow

    nc = bacc.Bacc("TRN2", target_bir_lowering=False)

    xT_d = nc.dram_tensor("xt", [D, N], bf16, kind="ExternalInput")
    xT8_d = nc.dram_tensor("xt8", [D, N], f8, kind="ExternalInput")
    wq_d = nc.dram_tensor("wq", [D, JW], bf16, kind="ExternalInput")
    wm_d = nc.dram_tensor("wm", [D, JW], bf16, kind="ExternalInput")
    yT_d = nc.dram_tensor("yt", [D, N], bf16, kind="ExternalOutput")

    with tile.TileContext(nc) as tc:
        with (
            tc.tile_pool(name="const", bufs=1) as cpool,
            tc.tile_pool(name="qt8", bufs=1) as qt8_pool,
            tc.tile_pool(name="exps", bufs=2) as exps_pool,
            tc.tile_pool(name="rcp", bufs=3) as rcp_pool,
            tc.tile_pool(name="mrg", bufs=5) as mrg_pool,
            tc.tile_pool(name="ps_stage", bufs=3, space="PSUM") as ps_stage,
            tc.tile_pool(name="ps_scores", bufs=3, space="PSUM") as ps_scores,
            tc.tile_pool(name="ps_av", bufs=2, space="PSUM") as ps_av,
        ):
            # ---- resident inputs ----
            xT = cpool.tile([128, KT, N], bf16, name="xT")
            xT8 = cpool.tile([128, KT, N], f8, name="xT8")
            wq = cpool.tile([128, KT, JW], bf16, name="wq")
            wm = cpool.tile([128, KT, JW], bf16, name="wm")
            # Each hwdge queue sustains only ~82GB/s, so the critical wave
            # is spread by need-time (the prologue interleaves heads 0/1
            # m-tile-wise, halving the xT consumption rate to ~76GB/s):
            #   sync:   xT cn-major (cn needed every ~6.9us)
            #   scalar: wm head 0 (first MM), then wq h-major fine slabs
            #           (head h needed at qt_phase(h))
            #   gpsimd: wm head 1 (~8us), wm head 2 (~35us), xT8 (~60us),
            #           wm head 3 (~65us)
            for cn in range(NCHUNK):
                for k in range(KT):
                    nc.sync.dma_start(
                        xT[:, k, cn * CW : (cn + 1) * CW],
                        xT_d[k * 128 : (k + 1) * 128, cn * CW : (cn + 1) * CW],
                    )
            for k in range(KT):
                nc.scalar.dma_start(
                    wm[:, k, 0:D], wm_d[k * 128 : (k + 1) * 128, 0:D]
                )
            for hh in range(HPG):
                for k in range(KT):
                    nc.scalar.dma_start(
                        wq[:, k, hh * D : (hh + 1) * D],
                        wq_d[k * 128 : (k + 1) * 128, hh * D : (hh + 1) * D],
                    )
            for h in (1, 2):
                for k in range(KT):
                    nc.gpsimd.dma_start(
                        wm[:, k, h * D : (h + 1) * D],
                        wm_d[k * 128 : (k + 1) * 128, h * D : (h + 1) * D],
                    )
            for k in range(KT):
                nc.gpsimd.dma_start(xT8[:, k, :], xT8_d[k * 128 : (k + 1) * 128, :])
            for k in range(KT):
                nc.gpsimd.dma_start(
                    wm[:, k, (HPG - 1) * D : HPG * D],
                    wm_d[k * 128 : (k + 1) * 128, (HPG - 1) * D : HPG * D],
                )

            # small PE warmup: ~12 dummy matmuls (~5us cold) give the DMA
            # queues a head start so the xM prologue never outruns delivery
            warm = cpool.tile([128, CW], bf16, name="warm")
            nc.vector.memset(warm[:, :], 1.0)
            ps_w = ps_stage.tile([128, CW], f32, name="ps_w", tag="stage")
            for _ in range(12):
                nc.tensor.matmul(
                    ps_w[:, :], lhsT=warm[:, 0:128], rhs=warm[:, :],
                    start=True, stop=True,
                )

            ones_col = cpool.tile([128, 1], bf16, name="ones_col")
            nc.vector.memset(ones_col[:, :], 1.0)
            # touch Exp once during the input-DMA wait so the ~2.7us ACT
            # table-set load is off the first chunk's critical path
            nc.scalar.activation(
                ones_col[:, :], ones_col[:, :],
                mybir.ActivationFunctionType.Exp, scale=0.0,
            )
            # f32r inputs to the sum+broadcast matmul must be produced by
            # "rounding" writes, so stage through a scratch tile
            f32r = mybir.dt.float32r
            ones128 = cpool.tile([128, 128], f32r, name="ones128")
            scr = mrg_pool.tile([128, CW], bf16, name="scr", tag="mrg")
            nc.vector.memset(scr[:, :], 1.0)
            nc.vector.tensor_copy(ones128[:, :], scr[:, 0:128])

            # per-head xM tiles (token-major values) and per-chunk y^T
            # accumulators
            xmn = [
                cpool.tile([128, NT, D], bf16, name=f"xmn{h}") for h in range(HPG)
            ]
            ysb = [
                cpool.tile([128, KT, CW], bf16, name=f"ysb{c}")
                for c in range(NCHUNK)
            ]

            # ---------- pipelined emission helpers ----------
            def xmn_block(h, mt):
                """xM_h token-major tile mt: 4 accumulation MMs + DVE cast."""
                ps = ps_stage.tile([128, D], f32, name="ps_x", tag="stage")
                for k in range(KT):
                    nc.tensor.matmul(
                        ps[:, :],
                        lhsT=xT[:, k, mt * 128 : (mt + 1) * 128],
                        rhs=wm[:, k, h * D : (h + 1) * D],
                        start=(k == 0),
                        stop=(k == KT - 1),
                    )
                nc.vector.tensor_copy(xmn[h][:, mt, :], ps[:, :])

            def av_block(p, i):
                """Slot i of 16: 4 AV accumulation MMs for pending chunk p
                (dt = i//4, m-tiles 4*(i%4)..+4); when a dt completes, merge
                the scaled result into the y^T accumulator (and DMA it out
                on the last head)."""
                if p is None:
                    return
                dt, m0 = i // 4, 4 * (i % 4)
                if m0 == 0:
                    p["ps"] = ps_av.tile([128, CW], f32, name="ps_av", tag="av")
                for mt in range(m0, m0 + 4):
                    nc.tensor.matmul(
                        p["ps"][:, :],
                        lhsT=p["xmn"][:, mt, dt * 128 : (dt + 1) * 128],
                        rhs=p["expS"][:, mt, :],
                        start=(mt == 0),
                        stop=(mt == NT - 1),
                    )
                if m0 + 4 == NT:
                    dst = ysb[p["c"]][:, dt, :]
                    if p["h"] == 0:
                        nc.vector.tensor_mul(dst, p["ps"][:, :], p["rcpB"][:, :])
                    elif p["h"] == HPG - 1 and p["c"] == NCHUNK - 1:
                        # final chunk: merge + DMA in halves so the
                        # end-of-kernel serial chain is ~0.7us shorter
                        t = mrg_pool.tile([128, CW], bf16, name="mrg", tag="mrg")
                        hw = CW // 2
                        for half in range(2):
                            lo, hi = half * hw, (half + 1) * hw
                            dsth = ysb[p["c"]][:, dt, lo:hi]
                            nc.vector.tensor_mul(
                                t[:, lo:hi], p["ps"][:, lo:hi], p["rcpB"][:, lo:hi]
                            )
                            nc.vector.tensor_add(dsth, dsth, t[:, lo:hi])
                            nc.sync.dma_start(
                                yT_d[dt * 128 : (dt + 1) * 128,
                                     p["n0"] + lo : p["n0"] + hi],
                                dsth,
                            )
                        return
                    else:
                        t = mrg_pool.tile([128, CW], bf16, name="mrg", tag="mrg")
                        nc.vector.tensor_mul(t[:, :], p["ps"][:, :], p["rcpB"][:, :])
                        nc.vector.tensor_add(dst, dst, t[:, :])
                    if p["h"] == HPG - 1:
                        nc.sync.dma_start(
                            yT_d[dt * 128 : (dt + 1) * 128,
                                 p["n0"] : p["n0"] + CW],
                            dst,
                        )

            def den_bcast(p):
                """Denominator finish for the pending chunk, emitted one
                slot into the NEXT phase: the 16 expS m-tiles were summed
                elementwise by a DVE running chain during p's own phase
                (p["zp"], f32r); one K=128 sum+broadcast matmul and the
                reciprocal produce rcpB."""
                if p is None or "zp" not in p:
                    return
                psb = ps_stage.tile([128, CW], f32, name="psb", tag="stage")
                nc.tensor.matmul(
                    psb[:, :], lhsT=ones128[:, :], rhs=p["zp"][:, :],
                    start=True, stop=True,
                )
                nc.vector.reciprocal_approx_fast(p["rcpB"][:, :], psb[:, :])
                del p["zp"]

            def qt_phase(h, pend):
                """q^T (fp8) for head h, computed directly on the PE as
                Wq_h^T x^T: 16 stages of 4 accumulation MMs + f32->fp8 DVE
                cast.  Runs bare (exp-independent, dense); the pending AV
                passes through to the following s_phase(h,0).  cn-major
                order so chunk 0's scores operands land first."""
                qT8 = qt8_pool.tile([128, KT, N], f8, name="qT8", tag="qT8")
                first = True
                for cn in range(NCHUNK):
                    for jb in range(KT):
                        ps = ps_stage.tile([128, CW], f32, name="ps_q", tag="stage")
                        for k in range(KT):
                            nc.tensor.matmul(
                                ps[:, :],
                                lhsT=wq[:, k, h * D + jb * 128 : h * D + (jb + 1) * 128],
                                rhs=xT[:, k, cn * CW : (cn + 1) * CW],
                                start=(k == 0),
                                stop=(k == KT - 1),
                            )
                        nc.vector.tensor_copy(qT8[:, jb, cn * CW : (cn + 1) * CW], ps[:, :])
                        if first:
                            den_bcast(pend)
                            first = False
                return qT8

            def s_phase(h, c, qT8, pend, filler=None):
                """Scores phase for chunk (h, c): 16 slots of [DR score
                pair + exp] interleaved with pend's AV; den groups for this
                chunk lag their exps by >=2 slots.  filler (s_phase(0,0)
                only, where no pending AV exists) emits real PE work to
                keep the slot structure dense."""
                n0 = c * CW
                expS = exps_pool.tile([128, NT, CW], bf16, name="expS", tag="expS")
                cur = {
                    "expS": expS, "xmn": xmn[h], "h": h, "c": c, "n0": n0,
                    "rcpB": rcp_pool.tile([128, CW], f32, name="rcpB", tag="rcpB"),
                }
                for mt in range(NT):
                    ps = ps_scores.tile([128, CW], f32, name="ps_s", tag="scores")
                    for t in range(KT // 2):
                        nc.tensor.matmul(
                            ps[:, :],
                            lhsT=xT8[:, 2 * t : 2 * t + 2, mt * 128 : (mt + 1) * 128],
                            rhs=qT8[:, 2 * t : 2 * t + 2, n0 : n0 + CW],
                            start=(t == 0),
                            stop=(t == KT // 2 - 1),
                            perf_mode=DR,
                        )
                    nc.scalar.activation(
                        expS[:, mt, :], ps[:, :],
                        mybir.ActivationFunctionType.Exp, scale=INV_SQRT_D,
                    )
                    # running elementwise sum of the expS m-tiles (DVE):
                    # replaces the packed PE column-sum matmuls
                    if mt == 1:
                        acc = mrg_pool.tile([128, CW], bf16, name="acc", tag="mrg")
                        cur["acc"] = acc
                        nc.vector.tensor_add(
                            acc[:, :], expS[:, 0, :], expS[:, 1, :]
                        )
                    elif mt >= 2:
                        acc = cur["acc"]
                        nc.vector.tensor_add(acc[:, :], acc[:, :], expS[:, mt, :])
                    av_block(pend, mt)
                    if filler is not None:
                        filler(mt)
                    if mt == 0:
                        den_bcast(pend)
                zp = rcp_pool.tile([128, CW], mybir.dt.float32r, name="zp", tag="zp")
                nc.vector.tensor_copy(zp[:, :], cur["acc"][:, :])
                cur["zp"] = zp
                return cur

            # ---------- the pipeline ----------
            # prologue: xM for heads 0..2 doubles as the HAM warmup (real
            # work from the first instruction); head 3's xM is s_phase(0,0)
            # filler.  Heads 0/1 interleave m-tile-wise so each xT column
            # chunk is consumed at half rate (one DMA queue keeps up).
            for mt in range(NT):
                xmn_block(0, mt)
                xmn_block(1, mt)
            for mt in range(NT):
                xmn_block(2, mt)
            pend = None
            for h in range(HPG):
                qT8 = qt_phase(h, pend)
                for c in range(NCHUNK):
                    filler = None
                    if h == 0 and c == 0:
                        filler = lambda mt: xmn_block(HPG - 1, mt)
                    cur = s_phase(h, c, qT8, pend, filler)
                    pend = cur
            # flush: last chunk's den + AV (merge DMAs the final y^T chunk)
            for i in range(NT):
                av_block(pend, i)
                if i == 0:
                    den_bcast(pend)

    nc.compile()
    return nc


def _ensure_nc():
    if "nc" not in _state:
        _state["nc"] = _build()
    return _state["nc"]


def _make_in_maps(x, Wq, Wp):
    bf = np.float16
    f8 = ml_dtypes.float8_e4m3
    # fold the output projection into per-head value matrices:
    # M_h = Wq_h @ Wp_h  (weight-only, input-independent)
    wms = []
    for hg in range(HG):
        Mi = np.empty((D, JW), np.float32)
        for hh in range(HPG):
            g = hg * HPG + hh
            Mi[:, hh * D : (hh + 1) * D] = (
                Wq[:, g * D : (g + 1) * D] @ Wp[g * D : (g + 1) * D, :]
            )
        wms.append(Mi.astype(bf))
    in_maps = []
    for c in range(NCORES):
        b, hg = c // HG, c % HG
        xt = np.ascontiguousarray(x[b].T)
        in_maps.append({
            "xt": xt.astype(bf),
            "xt8": xt.astype(f8),
            "wq": np.ascontiguousarray(Wq[:, hg * JW : (hg + 1) * JW]).astype(bf),
            "wm": wms[hg],
        })
    return in_maps


def _get_runner():
    """Build once and cache a jitted 8-core runner (avoids re-jit per call)."""
    if "run" in _state:
        return _state["run"]

    import jax
    import concourse.mybir as mybir
    from jax.sharding import Mesh, PartitionSpec
    from jax.experimental.shard_map import shard_map
    from concourse import bass2jax

    nc = _ensure_nc()
    bass2jax.install_neuronx_cc_hook()

    partition_name = nc.partition_id_tensor.name if nc.partition_id_tensor else None
    in_names, out_names, out_avals, zero_outs = [], [], [], []
    for alloc in nc.m.functions[0].allocations:
        if not isinstance(alloc, mybir.MemoryLocationSet):
            continue
        name = alloc.memorylocations[0].name
        if alloc.kind == "ExternalInput":
            if name != partition_name:
                in_names.append(name)
        elif alloc.kind == "ExternalOutput":
            shape = tuple(alloc.tensor_shape)
            dtype = mybir.dt.np(alloc.dtype)
            out_avals.append(jax.core.ShapedArray(shape, dtype))
            out_names.append(name)
            zero_outs.append(np.zeros(shape, dtype))
    n_params = len(in_names)
    n_outs = len(out_names)
    all_in_names = list(in_names) + list(out_names)
    if partition_name is not None:
        all_in_names.append(partition_name)

    def _body(*args):
        operands = list(args)
        if partition_name is not None:
            operands.append(bass2jax.partition_id_tensor())
        outs = bass2jax._bass_exec_p.bind(
            *operands,
            out_avals=tuple(out_avals),
            in_names=tuple(all_in_names),
            out_names=tuple(out_names),
            lowering_input_output_aliases=(),
            sim_require_finite=True,
            sim_require_nnan=True,
            nc=nc,
        )
        return tuple(outs)

    devices = jax.devices()[:NCORES]
    mesh = Mesh(np.asarray(devices), ("core",))
    in_specs = (PartitionSpec("core"),) * (n_params + n_outs)
    out_specs = (PartitionSpec("core"),) * n_outs
    sharded = jax.jit(
        shard_map(_body, mesh=mesh, in_specs=in_specs, out_specs=out_specs,
                  check_rep=False),
        donate_argnums=tuple(range(n_params, n_params + n_outs)),
        keep_unused=True,
    )

    def run(in_maps):
        concat_in = [
            np.concatenate([np.asarray(m[name]) for m in in_maps], axis=0)
            for name in in_names
        ]
        concat_zeros = [
            np.zeros((NCORES * z.shape[0], *z.shape[1:]), z.dtype) for z in zero_outs
        ]
        out_arrs = sharded(*concat_in, *concat_zeros)
        return [
            {
                name: np.asarray(out_arrs[i]).reshape(NCORES, *out_avals[i].shape)[c]
                for i, name in enumerate(out_names)
            }
            for c in range(NCORES)
        ]

    _state["run"] = run
    return run


def kernel(x, Wq, Wv, Wp, bp):
    x = np.asarray(x, np.float32)
    Wq = np.asarray(Wq, np.float32)
    Wp = np.asarray(Wp, np.float32)
    bp = np.asarray(bp, np.float32)

    run = _get_runner()
    results = run(_make_in_maps(x, Wq, Wp))
    y = np.empty((B, N, D), np.float32)
    for b in range(B):
        yt = (results[b * HG]["yt"].astype(np.float32)
              + results[b * HG + 1]["yt"].astype(np.float32))
        y[b] = yt.T + bp[None, :]
    return y


# revision 25
# speedup vs baseline: 1.1318x; 1.0024x over previous
"""Multi-head attention V2 kernel for Trainium2 (8 NeuronCores).

Problem shapes (hardcoded): x [4, 2048, 512] f32, Wq [512, 4096], Wv unused,
Wp [4096, 512], bp [512].  Reference math (note: V uses the Q projection):
    q = v = (x @ Wq) -> [B, H, N, D] with H=8, head dim = D = 512
    S = q @ x^T / sqrt(D);  P = softmax(S, -1);  out = (P @ v) @ Wp + bp

Sharding: core = (batch b, head-group hg) with 2 groups of 4 heads.

Weight folding (host): M_h = Wq_h @ Wp_h [D, D] per head, so
    y = sum_h P_h @ (x @ M_h) + bp
which eliminates the device-side output projection: per-head AV matmuls
use xM_h = x @ M_h as the value operand and their PSUM results merge
(scaled by the softmax reciprocal) into a single y^T accumulator in SBUF
on the DVE.  The xM_h tiles are computed on the PE in a prologue that
replaces the old dummy-matmul HAM warmup with real work.

q^T (fp8, scores rhs) is computed directly on the PE as Wq_h^T x^T
(lhsT = wq) and cast f32->fp8 by the DVE -- no token-major q, no DMA
xbar transposes, no SP-queue pressure, no head-boundary cast stalls.

The scores matmul S^T = x q^T runs in fp8e4 DoubleRow mode (K=256 per
instruction); both operands are TRN fp8_e4m3 (xT8 host-converted, qT8
cast on-device from f32 PSUM).  Everything else is fp16 with fp32 PSUM
accumulation; the y^T accumulator and output are fp16.

Pipelining: each s_phase interleaves chunk c's 16 [DR score pair + exp]
slots with the pending chunk's 64 AV matmuls (1 pair : 4 AV MMs per
slot, measured slot 1310ns).  The softmax denominator is a DVE running
elementwise sum of the expS m-tiles (off the PE); one K=128 f32r
sum+broadcast matmul + reciprocal at the top of the following phase
produce rcpB before the first AV merge needs it.  qt_phases (64 MMs,
exp-free) run bare between heads; s_phase(0,0), which has no pending
AV, interleaves head 3's xM tiles as filler.  Head 3's AV merges DMA
each completed y^T chunk straight out; the final chunk merges in halves
to shorten the end-of-kernel serial chain.  Input DMA is spread across
the sync/scalar/gpsimd hwdge queues by need-time (~82GB/s per queue).
"""

import sys

sys.path.insert(0, "/opt/trn_rl_repo")

import numpy as np
import ml_dtypes

B, N, D, H = 4, 2048, 512, 8
NCORES = 8
HG = 2            # head groups (cores per batch)
HPG = H // HG     # heads per core
JW = HPG * D      # per-core Wq column count (2048)
KT = D // 128     # k-tiles over feature dim (4)
NT = N // 128     # partition tiles over tokens (16)
NCHUNK = 4        # n split into 4 chunks of 512
CW = N // NCHUNK  # chunk width (512)
INV_SQRT_D = 1.0 / float(np.sqrt(D))

_state = {}


def _build():
    import concourse.bass as bass
    import concourse.mybir as mybir
    import concourse.tile as tile
    from concourse import bacc

    f32 = mybir.dt.float32
    bf16 = mybir.dt.float16
    f8 = mybir.dt.float8e4
    DR = mybir.MatmulPerfMode.DoubleRow

    nc = bacc.Bacc("TRN2", target_bir_lowering=False)

    xT_d = nc.dram_tensor("xt", [D, N], bf16, kind="ExternalInput")
    xT8_d = nc.dram_tensor("xt8", [D, N], f8, kind="ExternalInput")
    wq_d = nc.dram_tensor("wq", [D, JW], bf16, kind="ExternalInput")
    wm_d = nc.dram_tensor("wm", [D, JW], bf16, kind="ExternalInput")
    yT_d = nc.dram_tensor("yt", [D, N], bf16, kind="ExternalOutput")

    with tile.TileContext(nc) as tc:
        with (
            tc.tile_pool(name="const", bufs=1) as cpool,
            tc.tile_pool(name="qt8", bufs=1) as qt8_pool,
            tc.tile_pool(name="exps", bufs=2) as exps_pool,
            tc.tile_pool(name="rcp", bufs=3) as rcp_pool,
            tc.tile_pool(name="mrg", bufs=5) as mrg_pool,
            tc.tile_pool(name="ps_stage", bufs=3, space="PSUM") as ps_stage,
            tc.tile_pool(name="ps_scores", bufs=3, space="PSUM") as ps_scores,
            tc.tile_pool(name="ps_av", bufs=2, space="PSUM") as ps_av,
        ):
            # ---- resident inputs ----
            xT = cpool.tile([128, KT, N], bf16, name="xT")
            xT8 = cpool.tile([128, KT, N], f8, name="xT8")
            wq = cpool.tile([128, KT, JW], bf16, name="wq")
            wm = cpool.tile([128, KT, JW], bf16, name="wm")
            # Each hwdge queue sustains only ~110GB/s, so the critical
            # wave is spread by need-time (the prologue interleaves heads
            # 0/1 m-tile-wise; each xT chunk cn is needed at ~13.5+6.9*cn
            # us and lands with >5us margin):
            #   sync:   xT cn0, cn1
            #   scalar: wm head 0 (first MM), xT cn2, cn3, then wq h-major
            #           fine slabs (head h needed at qt_phase(h))
            #   gpsimd: wm head 1 (~14us), wm head 2 (~40us), xT8 (~75us),
            #           wm head 3 (~80us)
            for cn in (0, 1):
                for k in range(KT):
                    nc.sync.dma_start(
                        xT[:, k, cn * CW : (cn + 1) * CW],
                        xT_d[k * 128 : (k + 1) * 128, cn * CW : (cn + 1) * CW],
                    )
            for k in range(KT):
                nc.scalar.dma_start(
                    wm[:, k, 0:D], wm_d[k * 128 : (k + 1) * 128, 0:D]
                )
            for cn in (2, 3):
                for k in range(KT):
                    nc.scalar.dma_start(
                        xT[:, k, cn * CW : (cn + 1) * CW],
                        xT_d[k * 128 : (k + 1) * 128, cn * CW : (cn + 1) * CW],
                    )
            for hh in range(HPG):
                for k in range(KT):
                    nc.scalar.dma_start(
                        wq[:, k, hh * D : (hh + 1) * D],
                        wq_d[k * 128 : (k + 1) * 128, hh * D : (hh + 1) * D],
                    )
            for h in (1, 2):
                for k in range(KT):
                    nc.gpsimd.dma_start(
                        wm[:, k, h * D : (h + 1) * D],
                        wm_d[k * 128 : (k + 1) * 128, h * D : (h + 1) * D],
                    )
            for k in range(KT):
                nc.gpsimd.dma_start(xT8[:, k, :], xT8_d[k * 128 : (k + 1) * 128, :])
            for k in range(KT):
                nc.gpsimd.dma_start(
                    wm[:, k, (HPG - 1) * D : HPG * D],
                    wm_d[k * 128 : (k + 1) * 128, (HPG - 1) * D : HPG * D],
                )

            # small PE warmup: ~8 dummy matmuls (~3.4us cold) give the DMA
            # queues a head start so the xM prologue never outruns delivery
            warm = cpool.tile([128, CW], bf16, name="warm")
            nc.vector.memset(warm[:, :], 1.0)
            ps_w = ps_stage.tile([128, CW], f32, name="ps_w", tag="stage")
            for _ in range(8):
                nc.tensor.matmul(
                    ps_w[:, :], lhsT=warm[:, 0:128], rhs=warm[:, :],
                    start=True, stop=True,
                )

            ones_col = cpool.tile([128, 1], bf16, name="ones_col")
            nc.vector.memset(ones_col[:, :], 1.0)
            # touch Exp once during the input-DMA wait so the ~2.7us ACT
            # table-set load is off the first chunk's critical path
            nc.scalar.activation(
                ones_col[:, :], ones_col[:, :],
                mybir.ActivationFunctionType.Exp, scale=0.0,
            )
            # f32r inputs to the sum+broadcast matmul must be produced by
            # "rounding" writes, so stage through a scratch tile
            f32r = mybir.dt.float32r
            ones128 = cpool.tile([128, 128], f32r, name="ones128")
            scr = mrg_pool.tile([128, CW], bf16, name="scr", tag="mrg")
            nc.vector.memset(scr[:, :], 1.0)
            nc.vector.tensor_copy(ones128[:, :], scr[:, 0:128])

            # per-head xM tiles (token-major values) and per-chunk y^T
            # accumulators
            xmn = [
                cpool.tile([128, NT, D], bf16, name=f"xmn{h}") for h in range(HPG)
            ]
            ysb = [
                cpool.tile([128, KT, CW], bf16, name=f"ysb{c}")
                for c in range(NCHUNK)
            ]

            # ---------- pipelined emission helpers ----------
            def xmn_block(h, mt):
                """xM_h token-major tile mt: 4 accumulation MMs + DVE cast."""
                ps = ps_stage.tile([128, D], f32, name="ps_x", tag="stage")
                for k in range(KT):
                    nc.tensor.matmul(
                        ps[:, :],
                        lhsT=xT[:, k, mt * 128 : (mt + 1) * 128],
                        rhs=wm[:, k, h * D : (h + 1) * D],
                        start=(k == 0),
                        stop=(k == KT - 1),
                    )
                nc.vector.tensor_copy(xmn[h][:, mt, :], ps[:, :])

            def av_block(p, i):
                """Slot i of 16: 4 AV accumulation MMs for pending chunk p
                (dt = i//4, m-tiles 4*(i%4)..+4); when a dt completes, merge
                the scaled result into the y^T accumulator (and DMA it out
                on the last head)."""
                if p is None:
                    return
                dt, m0 = i // 4, 4 * (i % 4)
                if m0 == 0:
                    p["ps"] = ps_av.tile([128, CW], f32, name="ps_av", tag="av")
                for mt in range(m0, m0 + 4):
                    nc.tensor.matmul(
                        p["ps"][:, :],
                        lhsT=p["xmn"][:, mt, dt * 128 : (dt + 1) * 128],
                        rhs=p["expS"][:, mt, :],
                        start=(mt == 0),
                        stop=(mt == NT - 1),
                    )
                if m0 + 4 == NT:
                    dst = ysb[p["c"]][:, dt, :]
                    if p["h"] == 0:
                        nc.vector.tensor_mul(dst, p["ps"][:, :], p["rcpB"][:, :])
                    elif p["h"] == HPG - 1 and p["c"] == NCHUNK - 1:
                        # final chunk: merge + DMA in halves so the
                        # end-of-kernel serial chain is ~0.7us shorter
                        t = mrg_pool.tile([128, CW], bf16, name="mrg", tag="mrg")
                        hw = CW // 2
                        for half in range(2):
                            lo, hi = half * hw, (half + 1) * hw
                            dsth = ysb[p["c"]][:, dt, lo:hi]
                            nc.vector.tensor_mul(
                                t[:, lo:hi], p["ps"][:, lo:hi], p["rcpB"][:, lo:hi]
                            )
                            nc.vector.tensor_add(dsth, dsth, t[:, lo:hi])
                            nc.sync.dma_start(
                                yT_d[dt * 128 : (dt + 1) * 128,
                                     p["n0"] + lo : p["n0"] + hi],
                                dsth,
                            )
                        return
                    else:
                        t = mrg_pool.tile([128, CW], bf16, name="mrg", tag="mrg")
                        nc.vector.tensor_mul(t[:, :], p["ps"][:, :], p["rcpB"][:, :])
                        nc.vector.tensor_add(dst, dst, t[:, :])
                    if p["h"] == HPG - 1:
                        nc.sync.dma_start(
                            yT_d[dt * 128 : (dt + 1) * 128,
                                 p["n0"] : p["n0"] + CW],
                            dst,
                        )

            def den_bcast(p):
                """Denominator finish for the pending chunk, emitted one
                slot into the NEXT phase: the 16 expS m-tiles were summed
                elementwise by a DVE running chain during p's own phase
                (p["zp"], f32r); one K=128 sum+broadcast matmul and the
                reciprocal produce rcpB."""
                if p is None or "zp" not in p:
                    return
                psb = ps_stage.tile([128, CW], f32, name="psb", tag="stage")
                nc.tensor.matmul(
                    psb[:, :], lhsT=ones128[:, :], rhs=p["zp"][:, :],
                    start=True, stop=True,
                )
                nc.vector.reciprocal_approx_fast(p["rcpB"][:, :], psb[:, :])
                del p["zp"]

            def qt_phase(h, pend):
                """q^T (fp8) for head h, computed directly on the PE as
                Wq_h^T x^T: 16 stages of 4 accumulation MMs + f32->fp8 DVE
                cast.  Runs bare (exp-independent, dense); the pending AV
                passes through to the following s_phase(h,0).  cn-major
                order so chunk 0's scores operands land first."""
                qT8 = qt8_pool.tile([128, KT, N], f8, name="qT8", tag="qT8")
                first = True
                for cn in range(NCHUNK):
                    for jb in range(KT):
                        ps = ps_stage.tile([128, CW], f32, name="ps_q", tag="stage")
                        for k in range(KT):
                            nc.tensor.matmul(
                                ps[:, :],
                                lhsT=wq[:, k, h * D + jb * 128 : h * D + (jb + 1) * 128],
                                rhs=xT[:, k, cn * CW : (cn + 1) * CW],
                                start=(k == 0),
                                stop=(k == KT - 1),
                            )
                        nc.vector.tensor_copy(qT8[:, jb, cn * CW : (cn + 1) * CW], ps[:, :])
                        if first:
                            den_bcast(pend)
                            first = False
                return qT8

            def s_phase(h, c, qT8, pend, filler=None):
                """Scores phase for chunk (h, c): 16 slots of [DR score
                pair + exp] interleaved with pend's AV; den groups for this
                chunk lag their exps by >=2 slots.  filler (s_phase(0,0)
                only, where no pending AV exists) emits real PE work to
                keep the slot structure dense."""
                n0 = c * CW
                expS = exps_pool.tile([128, NT, CW], bf16, name="expS", tag="expS")
                cur = {
                    "expS": expS, "xmn": xmn[h], "h": h, "c": c, "n0": n0,
                    "rcpB": rcp_pool.tile([128, CW], f32, name="rcpB", tag="rcpB"),
                }
                for mt in range(NT):
                    ps = ps_scores.tile([128, CW], f32, name="ps_s", tag="scores")
                    for t in range(KT // 2):
                        nc.tensor.matmul(
                            ps[:, :],
                            lhsT=xT8[:, 2 * t : 2 * t + 2, mt * 128 : (mt + 1) * 128],
                            rhs=qT8[:, 2 * t : 2 * t + 2, n0 : n0 + CW],
                            start=(t == 0),
                            stop=(t == KT // 2 - 1),
                            perf_mode=DR,
                        )
                    nc.scalar.activation(
                        expS[:, mt, :], ps[:, :],
                        mybir.ActivationFunctionType.Exp, scale=INV_SQRT_D,
                    )
                    # running elementwise sum of the expS m-tiles (DVE):
                    # replaces the packed PE column-sum matmuls
                    if mt == 1:
                        acc = mrg_pool.tile([128, CW], bf16, name="acc", tag="mrg")
                        cur["acc"] = acc
                        nc.vector.tensor_add(
                            acc[:, :], expS[:, 0, :], expS[:, 1, :]
                        )
                    elif mt >= 2:
                        acc = cur["acc"]
                        nc.vector.tensor_add(acc[:, :], acc[:, :], expS[:, mt, :])
                    av_block(pend, mt)
                    if filler is not None:
                        filler(mt)
                    if mt == 0:
                        den_bcast(pend)
                zp = rcp_pool.tile([128, CW], mybir.dt.float32r, name="zp", tag="zp")
                nc.vector.tensor_copy(zp[:, :], cur["acc"][:, :])
                cur["zp"] = zp
                return cur

            # ---------- the pipeline ----------
            # prologue: xM for heads 0..2 doubles as the HAM warmup (real
            # work from the first instruction); head 3's xM is s_phase(0,0)
            # filler.  Heads 0/1 interleave m-tile-wise so each xT column
            # chunk is consumed at half rate (one DMA queue keeps up).
            for mt in range(NT):
                xmn_block(0, mt)
                xmn_block(1, mt)
            for mt in range(NT):
                xmn_block(2, mt)
            pend = None
            for h in range(HPG):
                qT8 = qt_phase(h, pend)
                for c in range(NCHUNK):
                    filler = None
                    if h == 0 and c == 0:
                        filler = lambda mt: xmn_block(HPG - 1, mt)
                    cur = s_phase(h, c, qT8, pend, filler)
                    pend = cur
            # flush: last chunk's den + AV (merge DMAs the final y^T chunk)
            for i in range(NT):
                av_block(pend, i)
                if i == 0:
                    den_bcast(pend)

    nc.compile()
    return nc


def _ensure_nc():
    if "nc" not in _state:
        _state["nc"] = _build()
    return _state["nc"]


def _make_in_maps(x, Wq, Wp):
    bf = np.float16
    f8 = ml_dtypes.float8_e4m3
    # fold the output projection into per-head value matrices:
    # M_h = Wq_h @ Wp_h  (weight-only, input-independent)
    wms = []
    for hg in range(HG):
        Mi = np.empty((D, JW), np.float32)
        for hh in range(HPG):
            g = hg * HPG + hh
            Mi[:, hh * D : (hh + 1) * D] = (
                Wq[:, g * D : (g + 1) * D] @ Wp[g * D : (g + 1) * D, :]
            )
        wms.append(Mi.astype(bf))
    in_maps = []
    for c in range(NCORES):
        b, hg = c // HG, c % HG
        xt = np.ascontiguousarray(x[b].T)
        in_maps.append({
            "xt": xt.astype(bf),
            "xt8": xt.astype(f8),
            "wq": np.ascontiguousarray(Wq[:, hg * JW : (hg + 1) * JW]).astype(bf),
            "wm": wms[hg],
        })
    return in_maps


def _get_runner():
    """Build once and cache a jitted 8-core runner (avoids re-jit per call)."""
    if "run" in _state:
        return _state["run"]

    import jax
    import concourse.mybir as mybir
    from jax.sharding import Mesh, PartitionSpec
    from jax.experimental.shard_map import shard_map
    from concourse import bass2jax

    nc = _ensure_nc()
    bass2jax.install_neuronx_cc_hook()

    partition_name = nc.partition_id_tensor.name if nc.partition_id_tensor else None
    in_names, out_names, out_avals, zero_outs = [], [], [], []
    for alloc in nc.m.functions[0].allocations:
        if not isinstance(alloc, mybir.MemoryLocationSet):
            continue
        name = alloc.memorylocations[0].name
        if alloc.kind == "ExternalInput":
            if name != partition_name:
                in_names.append(name)
        elif alloc.kind == "ExternalOutput":
            shape = tuple(alloc.tensor_shape)
            dtype = mybir.dt.np(alloc.dtype)
            out_avals.append(jax.core.ShapedArray(shape, dtype))
            out_names.append(name)
            zero_outs.append(np.zeros(shape, dtype))
    n_params = len(in_names)
    n_outs = len(out_names)
    all_in_names = list(in_names) + list(out_names)
    if partition_name is not None:
        all_in_names.append(partition_name)

    def _body(*args):
        operands = list(args)
        if partition_name is not None:
            operands.append(bass2jax.partition_id_tensor())
        outs = bass2jax._bass_exec_p.bind(
            *operands,
            out_avals=tuple(out_avals),
            in_names=tuple(all_in_names),
            out_names=tuple(out_names),
            lowering_input_output_aliases=(),
            sim_require_finite=True,
            sim_require_nnan=True,
            nc=nc,
        )
        return tuple(outs)

    devices = jax.devices()[:NCORES]
    mesh = Mesh(np.asarray(devices), ("core",))
    in_specs = (PartitionSpec("core"),) * (n_params + n_outs)
    out_specs = (PartitionSpec("core"),) * n_outs
    sharded = jax.jit(
        shard_map(_body, mesh=mesh, in_specs=in_specs, out_specs=out_specs,
                  check_rep=False),
        donate_argnums=tuple(range(n_params, n_params + n_outs)),
        keep_unused=True,
    )

    def run(in_maps):
        concat_in = [
            np.concatenate([np.asarray(m[name]) for m in in_maps], axis=0)
            for name in in_names
        ]
        concat_zeros = [
            np.zeros((NCORES * z.shape[0], *z.shape[1:]), z.dtype) for z in zero_outs
        ]
        out_arrs = sharded(*concat_in, *concat_zeros)
        return [
            {
                name: np.asarray(out_arrs[i]).reshape(NCORES, *out_avals[i].shape)[c]
                for i, name in enumerate(out_names)
            }
            for c in range(NCORES)
        ]

    _state["run"] = run
    return run


def kernel(x, Wq, Wv, Wp, bp):
    x = np.asarray(x, np.float32)
    Wq = np.asarray(Wq, np.float32)
    Wp = np.asarray(Wp, np.float32)
    bp = np.asarray(bp, np.float32)

    run = _get_runner()
    results = run(_make_in_maps(x, Wq, Wp))
    y = np.empty((B, N, D), np.float32)
    for b in range(B):
        yt = (results[b * HG]["yt"].astype(np.float32)
              + results[b * HG + 1]["yt"].astype(np.float32))
        y[b] = yt.T + bp[None, :]
    return y
